# revision 1
# baseline (speedup 1.0000x reference)
"""BitLinear 1.58-bit (nn_BitLinear158) Trainium2 kernel, 8-core tensor-parallel.

Math (must match reference):
  gamma_x = max(max|x|, eps); s = 128/gamma_x; xq = clip(round(x*s), -128, 127)
  gamma_w = max(mean|w|, eps); wq = clip(round(w/gamma_w), -1, 1)  (ternary)
  out = (xq @ wq.T) * (gamma_w / s) + bias

Key facts exploited:
  - xq in [-128,127] and wq in {-1,0,1} are exact in bf16; products and all
    PSUM partial sums are integers < 2^20, exact in fp32 => the GEMM runs at
    full bf16 PE rate and is bit-identical to the fp32 reference einsum.
  - wq = 1[w > 0.5*gamma_w] - 1[w < -0.5*gamma_w] (no division / round):
    round-half-even of w/gamma at +-0.5 and the clip at +-1.5 make the single
    threshold exact.
  - round-half-even via the fp32 magic constant 1.5*2^23 (valid for |v|<=2^22).

Sharding: weight/bias split over out_features (16384 -> 8 x 2048), x
replicated; gamma_w needs an AllReduce of per-shard |w| sums.
"""

from contextlib import ExitStack

import numpy as np

import concourse.bass as bass
import concourse.mybir as mybir
import concourse.tile as tile
from concourse import bass_utils
from concourse.masks import make_identity
from concourse.vector_clock import ScopedClock

# ---------------------------------------------------------------------------
# Workaround: this walrus build rejects instructions carrying >1-2 sync wait
# commands. Tile's tail drain (emitted after tile_legalize) aggregates one
# wait per outstanding proc onto a single InstDrain and so escapes the
# wait-count legalization. Redistribute its waits across a chain of NO-queue
# nops (same sequencer => program order preserves the barrier semantics).
# ---------------------------------------------------------------------------
_MAX_WAITS = 1


def _patched_drain_and_barrier(self, tick_clock, wait_clock):
    nc = self.nc
    probe = nc.sync.nop()
    wait_clock.add_sem_waits(probe.ins, ScopedClock({None: tick_clock.global_clock}))
    si = probe.ins.sync_info
    waits = list(si.on_wait) if si is not None and si.on_wait else []
    ups = list(si.on_update) if si is not None and si.on_update else []
    probe.ins.sync_info = mybir.SyncInfo(on_wait=waits[:_MAX_WAITS], on_update=ups)
    rest = waits[_MAX_WAITS:]
    while rest:
        n2 = nc.sync.nop()
        n2.ins.sync_info = mybir.SyncInfo(on_wait=rest[:_MAX_WAITS], on_update=[])
        rest = rest[_MAX_WAITS:]

    nc.sync.drain()

    nc.all_engine_barrier()
    assert self.sems is not None
    popped = nc._tile_sem_poison_stack.pop()
    assert popped is self._sem_poison
    nc.clear_and_free_semaphores(list(self.sems.allocated().values()))
    nc.all_engine_barrier()


tile.TileContext._drain_and_barrier = _patched_drain_and_barrier

_nop_counter = [0]


def _legalize_waits(nc):
    """Split >_MAX_WAITS sync waits per instruction onto same-engine nops
    inserted immediately before (per-engine program order => semantics kept)."""
    for f in nc.m.functions:
        for blk in f.blocks:
            out = []
            changed = False
            for inst in blk.instructions:
                si = getattr(inst, "sync_info", None)
                waits = list(si.on_wait) if si is not None and si.on_wait else []
                if len(waits) > _MAX_WAITS and inst.engine != mybir.EngineType.Unassigned:
                    while len(waits) > _MAX_WAITS:
                        chunk, waits = waits[:_MAX_WAITS], waits[_MAX_WAITS:]
                        _nop_counter[0] += 1
                        out.append(mybir.InstNoOp(
                            name=f"waitnop-{_nop_counter[0]}",
                            engine=inst.engine, ins=[], outs=[],
                            sync_info=mybir.SyncInfo(on_wait=chunk, on_update=[]),
                        ))
                    inst.sync_info = mybir.SyncInfo(
                        on_wait=waits,
                        on_update=list(si.on_update) if si.on_update else [])
                    changed = True
                out.append(inst)
            if changed:
                blk.instructions = out


# ---------------------------------------------------------------------------

N_CORES = 8
B, S, IN_F, OUT_F = 4, 2048, 4096, 16384
M = B * S                    # 8192 rows of x
N_SH = OUT_F // N_CORES      # 2048 output features per core
KT = IN_F // 128             # 32 k-tiles
MT = M // 128                # 64 m-tiles
NCH = N_SH // 512            # 4 psum column chunks
WT = N_SH // 128             # 16 weight row-tiles per core
EPS = 1e-5
MAGIC = 12582912.0           # 1.5 * 2^23: fp32 round-to-nearest-even trick
F32 = mybir.dt.float32
BF16 = mybir.dt.bfloat16

_CACHE = {}


M_SL = M // N_CORES  # per-core slice of x for the pass-1 max (1024 rows)


def _build(collective=True):
    nc = bass.Bass("TRN2", target_bir_lowering=False, debug=False,
                   num_devices=N_CORES if collective else 1)
    x_ap = nc.dram_tensor("x", [M, IN_F], F32, kind="ExternalInput").ap()
    x1_ap = nc.dram_tensor("x1", [M_SL, IN_F], F32, kind="ExternalInput").ap()
    w_ap = nc.dram_tensor("w", [N_SH, IN_F], F32, kind="ExternalInput").ap()
    b_ap = nc.dram_tensor("b", [1, N_SH], F32, kind="ExternalInput").ap()
    o_ap = nc.dram_tensor("o", [M, N_SH], F32, kind="ExternalOutput").ap()

    with tile.TileContext(nc) as tc:
        with ExitStack() as stack:
            _body(nc, tc, stack, x_ap, x1_ap, w_ap, b_ap, o_ap,
                  collective=collective)
    _legalize_waits(nc)
    return nc


def _body(nc, tc, stack, x_ap, x1_ap, w_ap, b_ap, o_ap, collective=True):
    def pool(name, bufs, space="SBUF"):
        return stack.enter_context(
            tc.tile_pool(name=name, bufs=bufs, space=space))

    # --- persistent SBUF ---
    wq_pool = pool("wq", 1)
    # wqT layout: [128 k-part, KT * N_SH] bf16, k-tile major
    wqT = wq_pool.tile([128, KT * N_SH], BF16, name="wqT", tag="wqT")
    const_pool = pool("const", 1)
    ident_bf = const_pool.tile([128, 128], BF16, name="ident_bf", tag="ibf")
    ident_f32 = const_pool.tile([128, 128], F32, name="ident_f32", tag="if32")
    ones_row = const_pool.tile([1, 128], F32, name="ones_row", tag="ones")
    bias_rep = const_pool.tile([128, N_SH], F32, name="bias_rep", tag="brep")
    scal128 = const_pool.tile([128, 4], F32, name="scal128", tag="scal128")
    magic128 = const_pool.tile([128, 1], F32, name="magic128", tag="magic")
    stats_pool = pool("stats", 1)
    wsums = stats_pool.tile([128, WT * 2], F32, name="wsums", tag="wsums")
    xmaxs = stats_pool.tile([128, MT * 2], F32, name="xmaxs", tag="xmaxs")
    stats2 = stats_pool.tile([128, 2], F32, name="stats2", tag="stats2")
    statsT_w = stats_pool.tile([1, 128], F32, name="statsT_w", tag="statsTw")
    statsT_x = stats_pool.tile([1, 128], F32, name="statsT_x", tag="statsTx")
    sc = stats_pool.tile([1, 8], F32, name="sc", tag="sc")

    # --- rotating SBUF ---
    io_pool = pool("io", 3)          # [128, 2048] f32 halves of x / w rows
    xq_pool = pool("xq", 1)          # [128, 4096] bf16 quantized row-tile
    xqT_pool = pool("xqT", 1)        # [128, 4096] bf16 transposed row-tile
    out_pool = pool("out", 1)        # [128, 2048] f32 staging
    bch_pool = pool("bch", 1)        # [1, 512] f32 bias chunks

    make_identity(nc, ident_bf[:])
    make_identity(nc, ident_f32[:])
    nc.gpsimd.memset(ones_row[:], 1.0)
    nc.gpsimd.memset(magic128[:], MAGIC)

    # PSUM pools: prep (2 banks) + pt (3) live together; po (5) opens after
    # prep closes => never more than 8 banks. pt opened first (stack order:
    # prep must close while pt stays open).
    pt_pool = tc.tile_pool(name="pt", bufs=3, space="PSUM")
    pt = pt_pool.__enter__()
    psum_prep = tc.tile_pool(name="psum_prep", bufs=2, space="PSUM")
    pp = psum_prep.__enter__()

    # ---------------- pass 1: |w| row sums + sliced max|x| ----------------
    # x responsibility for the global max is M-sharded across cores (each
    # core scans 1/8 of x = its x1 input); an AllReduce(max) recovers the
    # exact global max (max is exact under any order).
    for j in range(WT * 2):
        w_h = io_pool.tile([128, 2048], F32, name=f"wh_{j}", tag="io")
        nc.sync.dma_start(w_h[:], w_ap[(j // 2) * 128:(j // 2 + 1) * 128,
                                       (j % 2) * 2048:(j % 2 + 1) * 2048])
        nc.vector.tensor_reduce(wsums[:, j:j + 1], w_h[:],
                                axis=mybir.AxisListType.X,
                                op=mybir.AluOpType.add,
                                apply_absolute_value=True)
    nc.vector.tensor_reduce(stats2[:, 0:1], wsums[:],
                            axis=mybir.AxisListType.X, op=mybir.AluOpType.add)

    NX1 = (M_SL // 128) * 2  # 16 half-tiles of the x slice
    for j in range(NX1):
        x_h = io_pool.tile([128, 2048], F32, name=f"xh1_{j}", tag="io")
        nc.sync.dma_start(x_h[:], x1_ap[(j // 2) * 128:(j // 2 + 1) * 128,
                                        (j % 2) * 2048:(j % 2 + 1) * 2048])
        nc.vector.tensor_reduce(xmaxs[:, j:j + 1], x_h[:],
                                axis=mybir.AxisListType.X,
                                op=mybir.AluOpType.max,
                                apply_absolute_value=True)
    nc.vector.tensor_reduce(stats2[:, 1:2], xmaxs[:, 0:NX1],
                            axis=mybir.AxisListType.X, op=mybir.AluOpType.max)

    # cross-partition reductions via PE transpose
    st_ps_w = pp.tile([1, 128], F32, name="st_ps_w", tag="prep")
    nc.tensor.transpose(st_ps_w[:], stats2[:, 0:1], ident_f32[:])
    nc.vector.tensor_copy(statsT_w[:], st_ps_w[:])
    nc.vector.tensor_reduce(sc[0:1, 4:5], statsT_w[:],
                            axis=mybir.AxisListType.X, op=mybir.AluOpType.add)
    st_ps_x = pp.tile([1, 128], F32, name="st_ps_x", tag="prep")
    nc.tensor.transpose(st_ps_x[:], stats2[:, 1:2], ident_f32[:])
    nc.vector.tensor_copy(statsT_x[:], st_ps_x[:])
    nc.vector.tensor_reduce(sc[0:1, 5:6], statsT_x[:],
                            axis=mybir.AxisListType.X, op=mybir.AluOpType.max)

    if collective:
        dram_pool = pool("dram", 1, space="DRAM")
        cc_in = dram_pool.tile([1, 2], F32, name="cc_in", tag="cc_in")
        cc_out_s = dram_pool.tile([1, 1], F32, name="cc_out_s", tag="cc_out_s",
                                  addr_space="Shared")
        cc_out_m = dram_pool.tile([1, 1], F32, name="cc_out_m", tag="cc_out_m",
                                  addr_space="Shared")
        nc.gpsimd.dma_start(cc_in[:], sc[0:1, 4:6])
        nc.gpsimd.collective_compute(
            "AllReduce", mybir.AluOpType.max,
            replica_groups=[list(range(N_CORES))],
            ins=[cc_in[0:1, 1:2].opt()], outs=[cc_out_m.opt()],
        )
        nc.gpsimd.collective_compute(
            "AllReduce", mybir.AluOpType.add,
            replica_groups=[list(range(N_CORES))],
            ins=[cc_in[0:1, 0:1].opt()], outs=[cc_out_s.opt()],
        )
        nc.gpsimd.dma_start(sc[0:1, 6:7], cc_out_s[:])
        nc.gpsimd.dma_start(sc[0:1, 5:6], cc_out_m[:])
        wsum_all = sc[0:1, 6:7]
        inv_cnt = 1.0 / (OUT_F * IN_F)
    else:  # single-core sim variant: local stats stand in for global ones
        wsum_all = sc[0:1, 4:5]
        inv_cnt = 1.0 / (N_SH * IN_F)

    # gamma_w = max(sum/count, eps)  -> sc[0,7]
    nc.vector.tensor_scalar(sc[0:1, 7:8], wsum_all,
                            inv_cnt, EPS,
                            op0=mybir.AluOpType.mult, op1=mybir.AluOpType.max)
    # thr = 0.5*gamma_w -> sc[0,2]; nthr -> sc[0,3]
    nc.vector.tensor_scalar(sc[0:1, 2:3], sc[0:1, 7:8], 0.5, None,
                            op0=mybir.AluOpType.mult)
    nc.vector.tensor_scalar(sc[0:1, 3:4], sc[0:1, 7:8], -0.5, None,
                            op0=mybir.AluOpType.mult)
    scw_ps = pp.tile([128, 2], F32, name="scw_ps", tag="prep")
    nc.tensor.matmul(scw_ps[:], ones_row[:], sc[0:1, 2:4], start=True, stop=True)
    nc.vector.tensor_copy(scal128[:, 2:4], scw_ps[:])
    thr128 = scal128[:, 2:3]
    nthr128 = scal128[:, 3:4]

    # -------- quantize + transpose the weight shard --------
    for r in range(WT):
        wq_t = xq_pool.tile([128, IN_F], BF16, name=f"wqt_{r}", tag="xq")
        for h in range(2):
            w_h = io_pool.tile([128, 2048], F32, name=f"wh2_{r}_{h}", tag="io")
            nc.sync.dma_start(w_h[:], w_ap[r * 128:(r + 1) * 128,
                                           h * 2048:(h + 1) * 2048])
            neg = out_pool.tile([128, 2048], F32, name=f"neg_{r}_{h}", tag="out")
            nc.gpsimd.tensor_scalar(neg[:], w_h[:], nthr128, None,
                                    op0=mybir.AluOpType.is_lt)
            # wq = (w > thr) - (w < -thr)   in {-1, 0, 1}, bf16
            nc.vector.scalar_tensor_tensor(
                wq_t[:, h * 2048:(h + 1) * 2048], w_h[:], thr128, neg[:],
                op0=mybir.AluOpType.is_gt, op1=mybir.AluOpType.subtract)
        for k in range(KT):
            ptt = pt.tile([128, 128], BF16, name=f"wpt_{r}_{k}", tag="pt")
            nc.tensor.transpose(ptt[:], wq_t[:, k * 128:(k + 1) * 128],
                                ident_bf[:])
            nc.any.tensor_copy(wqT[:, k * N_SH + r * 128: k * N_SH + (r + 1) * 128],
                               ptt[:])

    # gamma_x = max(global max, eps) in place of sc[0,5]
    nc.vector.tensor_scalar(sc[0:1, 5:6], sc[0:1, 5:6], EPS, None,
                            op0=mybir.AluOpType.max)
    # scale_x = 128 * (1/gamma_x) -> sc[0,0]
    nc.vector.reciprocal(sc[0:1, 0:1], sc[0:1, 5:6])
    nc.vector.tensor_scalar(sc[0:1, 0:1], sc[0:1, 0:1], 128.0, None,
                            op0=mybir.AluOpType.mult)
    # r = gamma_w * gamma_x / 128 -> sc[0,1]
    nc.vector.tensor_scalar(sc[0:1, 1:2], sc[0:1, 5:6], 1.0 / 128.0, None,
                            op0=mybir.AluOpType.mult)
    nc.vector.tensor_mul(sc[0:1, 1:2], sc[0:1, 1:2], sc[0:1, 7:8])
    scx_ps = pp.tile([128, 2], F32, name="scx_ps", tag="prep")
    nc.tensor.matmul(scx_ps[:], ones_row[:], sc[0:1, 0:2], start=True, stop=True)
    nc.vector.tensor_copy(scal128[:, 0:2], scx_ps[:])
    scale128 = scal128[:, 0:1]
    r128 = scal128[:, 1:2]

    # bias broadcast to 128 partitions
    for n in range(NCH):
        bch = bch_pool.tile([1, 512], F32, name=f"bch_{n}", tag="bch")
        nc.sync.dma_start(bch[:], b_ap[0:1, n * 512:(n + 1) * 512])
        b_ps = pp.tile([128, 512], F32, name=f"b_ps_{n}", tag="prep")
        nc.tensor.matmul(b_ps[:], ones_row[:], bch[:], start=True, stop=True)
        nc.vector.tensor_copy(bias_rep[:, n * 512:(n + 1) * 512], b_ps[:])
    psum_prep.__exit__(None, None, None)

    po_pool = tc.tile_pool(name="po", bufs=5, space="PSUM")
    po = po_pool.__enter__()

    # ---------------- main loop over m-tiles ----------------
    for i in range(MT):
        xq_t = xq_pool.tile([128, IN_F], BF16, name=f"xq_{i}", tag="xq")
        for h in range(2):
            x_h = io_pool.tile([128, 2048], F32, name=f"xh2_{i}_{h}", tag="io")
            nc.sync.dma_start(x_h[:], x_ap[i * 128:(i + 1) * 128,
                                           h * 2048:(h + 1) * 2048])
            # xs = round_to_int(x*s), in place: magic-add rounds half-to-even
            nc.scalar.activation(x_h[:], x_h[:],
                                 mybir.ActivationFunctionType.Identity,
                                 bias=magic128[:], scale=scale128)
            # xq = min(xs - magic, 127) -> bf16 (>= -128 by construction)
            nc.vector.tensor_scalar(xq_t[:, h * 2048:(h + 1) * 2048], x_h[:],
                                    MAGIC, 127.0,
                                    op0=mybir.AluOpType.subtract,
                                    op1=mybir.AluOpType.min)

        xqT_t = xqT_pool.tile([128, IN_F], BF16, name=f"xqT_{i}", tag="xqT")
        pous = [po.tile([128, 512], F32, name=f"po_{i}_{n}", tag="po")
                for n in range(NCH)]

        def transpose_k(k):
            ptt = pt.tile([128, 128], BF16, name=f"xpt_{i}_{k}", tag="pt")
            nc.tensor.transpose(ptt[:], xq_t[:, k * 128:(k + 1) * 128],
                                ident_bf[:])
            nc.any.tensor_copy(xqT_t[:, k * 128:(k + 1) * 128], ptt[:])

        # software-pipelined: T(k) runs on PE between MM(k-1) bursts
        transpose_k(0)
        transpose_k(1)
        for k in range(KT):
            for n in range(NCH):
                nc.tensor.matmul(
                    pous[n][:],
                    xqT_t[:, k * 128:(k + 1) * 128],
                    wqT[:, k * N_SH + n * 512: k * N_SH + (n + 1) * 512],
                    start=(k == 0), stop=(k == KT - 1))
            if k + 2 < KT:
                transpose_k(k + 2)

        o_t = out_pool.tile([128, N_SH], F32, name=f"ot_{i}", tag="out")
        for n in range(NCH):
            # out = psum * r + bias
            nc.vector.scalar_tensor_tensor(
                o_t[:, n * 512:(n + 1) * 512], pous[n][:], r128,
                bias_rep[:, n * 512:(n + 1) * 512],
                op0=mybir.AluOpType.mult, op1=mybir.AluOpType.add)
        nc.sync.dma_start(o_ap[i * 128:(i + 1) * 128, :], o_t[:])

    po_pool.__exit__(None, None, None)
    pt_pool.__exit__(None, None, None)


def kernel(**inputs):
    x = np.ascontiguousarray(inputs["input"], dtype=np.float32).reshape(M, IN_F)
    w = np.ascontiguousarray(inputs["weight"], dtype=np.float32)
    b = np.ascontiguousarray(inputs["bias"], dtype=np.float32)

    if "nc" not in _CACHE:
        _CACHE["nc"] = _build()
    nc = _CACHE["nc"]

    in_maps = []
    for c in range(N_CORES):
        in_maps.append({
            "x": x,
            "x1": x[c * M_SL:(c + 1) * M_SL],
            "w": w[c * N_SH:(c + 1) * N_SH],
            "b": b[c * N_SH:(c + 1) * N_SH].reshape(1, N_SH),
        })
    res = bass_utils.run_bass_kernel_spmd(nc, in_maps,
                                          core_ids=list(range(N_CORES)))
    _CACHE["last_results"] = res
    out = np.concatenate([r["o"] for r in res.results], axis=1)
    return out.reshape(B, S, OUT_F)



# revision 4
# speedup vs baseline: 1.0408x; 1.0408x over previous
"""BitLinear 1.58-bit (nn_BitLinear158) Trainium2 kernel, 8-core tensor-parallel.

Math (must match reference):
  gamma_x = max(max|x|, eps); s = 128/gamma_x; xq = clip(round(x*s), -128, 127)
  gamma_w = max(mean|w|, eps); wq = clip(round(w/gamma_w), -1, 1)  (ternary)
  out = (xq @ wq.T) * (gamma_w / s) + bias

Key facts exploited:
  - xq in [-128,127] and wq in {-1,0,1} are exact in bf16; products and all
    PSUM partial sums are integers < 2^20, exact in fp32 => the GEMM runs at
    full bf16 PE rate and is bit-identical to the fp32 reference einsum.
  - wq = 1[w > 0.5*gamma_w] - 1[w < -0.5*gamma_w] (no division / round).
  - round-half-even via the fp32 magic constant 1.5*2^23 (valid for |v|<=2^22).

v2 vs baseline:
  - x and w are handed over TRANSPOSED (k-major) by the host wrapper, so
    both GEMM operands arrive with the contraction dim on partitions and the
    PE never runs a transpose: it executes matmuls only.
  - Redundant InstLdweights (same stationary tile as the previous matmul)
    are rewritten to no-ops post-scheduling: 4 matmuls (n-chunks) share one
    weight load.
  - One AllGather replaces the two AllReduces for (sum|w|, max|x|).
  - All 8 PSUM banks double-buffer the accumulation groups.

Sharding: weight/bias split over out_features (16384 -> 8 x 2048), x
replicated; per-core GEMM [8192,4096]x[4096,2048].
"""

from contextlib import ExitStack

import numpy as np

import concourse.bass as bass
import concourse.mybir as mybir
import concourse.tile as tile
from concourse import bass_utils
from concourse.masks import make_identity
from concourse.vector_clock import ScopedClock

# ---------------------------------------------------------------------------
# Workaround: this walrus build rejects instructions carrying >1-2 sync wait
# commands. Tile's tail drain (emitted after tile_legalize) aggregates one
# wait per outstanding proc onto a single InstDrain and so escapes the
# wait-count legalization. Redistribute its waits across a chain of NO-queue
# nops (same sequencer => program order preserves the barrier semantics).
# ---------------------------------------------------------------------------
_MAX_WAITS = 1


def _patched_drain_and_barrier(self, tick_clock, wait_clock):
    nc = self.nc
    probe = nc.sync.nop()
    wait_clock.add_sem_waits(probe.ins, ScopedClock({None: tick_clock.global_clock}))
    si = probe.ins.sync_info
    waits = list(si.on_wait) if si is not None and si.on_wait else []
    ups = list(si.on_update) if si is not None and si.on_update else []
    probe.ins.sync_info = mybir.SyncInfo(on_wait=waits[:_MAX_WAITS], on_update=ups)
    rest = waits[_MAX_WAITS:]
    while rest:
        n2 = nc.sync.nop()
        n2.ins.sync_info = mybir.SyncInfo(on_wait=rest[:_MAX_WAITS], on_update=[])
        rest = rest[_MAX_WAITS:]

    nc.sync.drain()

    nc.all_engine_barrier()
    assert self.sems is not None
    popped = nc._tile_sem_poison_stack.pop()
    assert popped is self._sem_poison
    nc.clear_and_free_semaphores(list(self.sems.allocated().values()))
    nc.all_engine_barrier()


tile.TileContext._drain_and_barrier = _patched_drain_and_barrier

_nop_counter = [0]


def _legalize_waits(nc):
    """Split >_MAX_WAITS sync waits per instruction onto same-engine nops
    inserted immediately before (per-engine program order => semantics kept)."""
    for f in nc.m.functions:
        for blk in f.blocks:
            out = []
            changed = False
            for inst in blk.instructions:
                si = getattr(inst, "sync_info", None)
                waits = list(si.on_wait) if si is not None and si.on_wait else []
                if len(waits) > _MAX_WAITS and inst.engine != mybir.EngineType.Unassigned:
                    while len(waits) > _MAX_WAITS:
                        chunk, waits = waits[:_MAX_WAITS], waits[_MAX_WAITS:]
                        _nop_counter[0] += 1
                        out.append(mybir.InstNoOp(
                            name=f"waitnop-{_nop_counter[0]}",
                            engine=inst.engine, ins=[], outs=[],
                            sync_info=mybir.SyncInfo(on_wait=chunk, on_update=[]),
                        ))
                    inst.sync_info = mybir.SyncInfo(
                        on_wait=waits,
                        on_update=list(si.on_update) if si.on_update else [])
                    changed = True
                out.append(inst)
            if changed:
                blk.instructions = out


def _ldw_key(inst):
    ap = inst.ins[0]
    return (str(ap.ap), ap.offset, str(ap.dtype), ap.memref)


def _dedup_ldweights(nc, verbose=False):
    """Rewrite InstLdweights that reload the stationary tile already resident
    in the PE array into no-ops (PE weight regs persist across matmuls; only
    transpose-mode matmuls clobber them)."""
    total = dropped = 0
    for f in nc.m.functions:
        for blk in f.blocks:
            out = []
            last = None
            changed = False
            for inst in blk.instructions:
                if inst.engine == mybir.EngineType.PE:
                    tn = type(inst).__name__
                    if tn == "InstLdweights":
                        total += 1
                        key = _ldw_key(inst)
                        if key == last:
                            dropped += 1
                            changed = True
                            si = inst.sync_info
                            has_sync = si is not None and (si.on_wait or si.on_update)
                            if has_sync:
                                _nop_counter[0] += 1
                                out.append(mybir.InstNoOp(
                                    name=f"ldwnop-{_nop_counter[0]}",
                                    engine=mybir.EngineType.PE, ins=[], outs=[],
                                    sync_info=si))
                            continue
                        last = key
                    elif tn == "InstMatmult":
                        if getattr(inst, "is_transpose", False):
                            last = None
                    elif tn in ("InstNoOp", "InstEventSemaphore", "InstDrain",
                                "InstRegisterMove", "InstUnconditionalBranch"):
                        pass
                    else:
                        last = None
                out.append(inst)
            if changed:
                blk.instructions = out
    if verbose:
        print(f"_dedup_ldweights: dropped {dropped}/{total}")
    return dropped, total


# ---------------------------------------------------------------------------

N_CORES = 8
B, S, IN_F, OUT_F = 4, 2048, 4096, 16384
M = B * S                    # 8192 rows of x
N_SH = OUT_F // N_CORES      # 2048 output features per core
M_SL = M // N_CORES          # per-core slice of x for the pass-1 max
EPS = 1e-5
MAGIC = 12582912.0           # 1.5 * 2^23: fp32 round-to-nearest-even trick
F32 = mybir.dt.float32
BF16 = mybir.dt.bfloat16

_CACHE = {}


def _build(collective=True, m=M, in_f=IN_F, n_sh=N_SH, m_sl=M_SL, out_f=OUT_F,
           postpasses=True):
    nc = bass.Bass("TRN2", target_bir_lowering=False, debug=False,
                   num_devices=N_CORES if collective else 1)
    xt_ap = nc.dram_tensor("xt", [in_f, m], F32, kind="ExternalInput").ap()
    x1_ap = nc.dram_tensor("x1", [in_f, m_sl], F32, kind="ExternalInput").ap()
    w_ap = nc.dram_tensor("w", [in_f, n_sh], F32, kind="ExternalInput").ap()
    b_ap = nc.dram_tensor("b", [1, n_sh], F32, kind="ExternalInput").ap()
    o_ap = nc.dram_tensor("o", [m, n_sh], F32, kind="ExternalOutput").ap()

    with tile.TileContext(nc) as tc:
        with ExitStack() as stack:
            _body(nc, tc, stack, xt_ap, x1_ap, w_ap, b_ap, o_ap,
                  collective=collective, m=m, in_f=in_f, n_sh=n_sh,
                  m_sl=m_sl, out_f=out_f)
    if postpasses:
        _dedup_ldweights(nc, verbose=True)
        _legalize_waits(nc)
    return nc


def _body(nc, tc, stack, xt_ap, x1_ap, w_ap, b_ap, o_ap, collective,
          m, in_f, n_sh, m_sl, out_f):
    KT = in_f // 128             # k-tiles
    MT = m // 128                # m-tiles
    NCH = n_sh // 512            # psum column chunks per m-tile
    KG = min(16, KT)             # k-tiles per staging DMA
    NKG = KT // KG               # staging DMAs per m-tile

    def pool(name, bufs, space="SBUF"):
        return stack.enter_context(
            tc.tile_pool(name=name, bufs=bufs, space=space))

    # --- persistent SBUF ---
    wq_pool = pool("wq", 1)
    # wqT layout: [128 k-part, KT * n_sh] bf16, k-tile major
    wqT = wq_pool.tile([128, KT * n_sh], BF16, name="wqT", tag="wqT")
    const_pool = pool("const", 1)
    ident_f32 = const_pool.tile([128, 128], F32, name="ident_f32", tag="if32")
    ones_row = const_pool.tile([1, 128], F32, name="ones_row", tag="ones")
    bias_rep = const_pool.tile([128, n_sh], F32, name="bias_rep", tag="brep")
    scal128 = const_pool.tile([128, 4], F32, name="scal128", tag="scal128")
    magic128 = const_pool.tile([128, 1], F32, name="magic128", tag="magic")
    stats_pool = pool("stats", 1)
    wsums = stats_pool.tile([128, KT], F32, name="wsums", tag="wsums")
    xmaxs = stats_pool.tile([128, KT], F32, name="xmaxs", tag="xmaxs")
    stats2 = stats_pool.tile([128, 2], F32, name="stats2", tag="stats2")
    statsT_w = stats_pool.tile([1, 128], F32, name="statsT_w", tag="statsTw")
    statsT_x = stats_pool.tile([1, 128], F32, name="statsT_x", tag="statsTx")
    sc = stats_pool.tile([1, 12], F32, name="sc", tag="sc")
    ag = stats_pool.tile([1, 2 * N_CORES], F32, name="ag", tag="ag")

    # --- rotating SBUF ---
    io_pool = pool("io", 3)          # [128, 2048] f32 staging (w / x slabs)
    neg_pool = pool("neg", 2)        # [128, 2048] f32 scratch for w quantize
    xq_pool = pool("xq", 2)          # [128, KT*128] bf16 quantized m-slab
    out_pool = pool("outp", 2)       # [128, 1024] f32 output staging
    bch_pool = pool("bch", 2)        # [1, 512] f32 bias chunks

    make_identity(nc, ident_f32[:])
    nc.gpsimd.memset(ones_row[:], 1.0)
    nc.gpsimd.memset(magic128[:], MAGIC)

    psum_prep = tc.tile_pool(name="psum_prep", bufs=2, space="PSUM")
    pp = psum_prep.__enter__()

    # ---------------- pass 1: |w| row sums + sliced max|x| ----------------
    # x responsibility for the global max is M-sharded across cores (each
    # core scans 1/8 of x = its x1 input, columns of xT); an AllReduce(max)
    # recovers the exact global max.
    for j in range(KT):
        w_h = io_pool.tile([128, 2048], F32, name=f"wh_{j}", tag="io")
        nc.sync.dma_start(w_h[:, 0:n_sh], w_ap[j * 128:(j + 1) * 128, :])
        nc.vector.tensor_reduce(wsums[:, j:j + 1], w_h[:, 0:n_sh],
                                axis=mybir.AxisListType.X,
                                op=mybir.AluOpType.add,
                                apply_absolute_value=True)
    nc.vector.tensor_reduce(stats2[:, 0:1], wsums[:],
                            axis=mybir.AxisListType.X, op=mybir.AluOpType.add)

    for j in range(KT // 2):
        x_h = io_pool.tile([128, 2048], F32, name=f"xh1_{j}", tag="io")
        nc.sync.dma_start(x_h[:, 0:m_sl],
                          x1_ap[(2 * j) * 128:(2 * j + 1) * 128, :])
        nc.sync.dma_start(x_h[:, m_sl:2 * m_sl],
                          x1_ap[(2 * j + 1) * 128:(2 * j + 2) * 128, :])
        nc.vector.tensor_reduce(xmaxs[:, j:j + 1], x_h[:, 0:2 * m_sl],
                                axis=mybir.AxisListType.X,
                                op=mybir.AluOpType.max,
                                apply_absolute_value=True)
    nc.vector.tensor_reduce(stats2[:, 1:2], xmaxs[:, 0:KT // 2],
                            axis=mybir.AxisListType.X, op=mybir.AluOpType.max)

    # cross-partition reductions via PE transpose
    st_ps_w = pp.tile([1, 128], F32, name="st_ps_w", tag="prep")
    nc.tensor.transpose(st_ps_w[:], stats2[:, 0:1], ident_f32[:])
    nc.vector.tensor_copy(statsT_w[:], st_ps_w[:])
    nc.vector.tensor_reduce(sc[0:1, 0:1], statsT_w[:],
                            axis=mybir.AxisListType.X, op=mybir.AluOpType.add)
    st_ps_x = pp.tile([1, 128], F32, name="st_ps_x", tag="prep")
    nc.tensor.transpose(st_ps_x[:], stats2[:, 1:2], ident_f32[:])
    nc.vector.tensor_copy(statsT_x[:], st_ps_x[:])
    nc.vector.tensor_reduce(sc[0:1, 1:2], statsT_x[:],
                            axis=mybir.AxisListType.X, op=mybir.AluOpType.max)

    if collective:
        dram_pool = pool("dram", 1, space="DRAM")
        cc_in = dram_pool.tile([1, 2], F32, name="cc_in", tag="cc_in")
        cc_out = dram_pool.tile([1, 2 * N_CORES], F32, name="cc_out",
                                tag="cc_out", addr_space="Shared")
        nc.gpsimd.dma_start(cc_in[:], sc[0:1, 0:2])
        nc.gpsimd.collective_compute(
            "AllGather", mybir.AluOpType.bypass,
            replica_groups=[list(range(N_CORES))],
            ins=[cc_in[:].opt()], outs=[cc_out[:].opt()],
        )
        nc.gpsimd.dma_start(ag[:], cc_out[:])
        # core-major [w0, x0, w1, x1, ...] -> strided views
        ag3 = ag[:].rearrange("p (c t) -> p t c", t=2)
        nc.vector.tensor_reduce(sc[0:1, 2:3], ag3[0:1, 0:1, :],
                                axis=mybir.AxisListType.X,
                                op=mybir.AluOpType.add)
        nc.vector.tensor_reduce(sc[0:1, 3:4], ag3[0:1, 1:2, :],
                                axis=mybir.AxisListType.X,
                                op=mybir.AluOpType.max)
        wsum_all = sc[0:1, 2:3]
        xmax_all = sc[0:1, 3:4]
        inv_cnt = 1.0 / (out_f * in_f)
    else:  # single-core sim variant: local stats stand in for global ones
        wsum_all = sc[0:1, 0:1]
        xmax_all = sc[0:1, 1:2]
        inv_cnt = 1.0 / (n_sh * in_f)

    # gamma_w = max(sum/count, eps) -> sc[0,8]
    nc.vector.tensor_scalar(sc[0:1, 8:9], wsum_all, inv_cnt, EPS,
                            op0=mybir.AluOpType.mult, op1=mybir.AluOpType.max)
    # thr = 0.5*gamma_w -> sc[0,4]; nthr -> sc[0,5]
    nc.vector.tensor_scalar(sc[0:1, 4:5], sc[0:1, 8:9], 0.5, None,
                            op0=mybir.AluOpType.mult)
    nc.vector.tensor_scalar(sc[0:1, 5:6], sc[0:1, 8:9], -0.5, None,
                            op0=mybir.AluOpType.mult)
    # gamma_x = max(xmax, eps) -> sc[0,3] slot
    nc.vector.tensor_scalar(sc[0:1, 3:4], xmax_all, EPS, None,
                            op0=mybir.AluOpType.max)
    # scale_x = 128 / gamma_x -> sc[0,6]
    nc.vector.reciprocal(sc[0:1, 6:7], sc[0:1, 3:4])
    nc.vector.tensor_scalar(sc[0:1, 6:7], sc[0:1, 6:7], 128.0, None,
                            op0=mybir.AluOpType.mult)
    # r = gamma_w * gamma_x / 128 -> sc[0,7]
    nc.vector.tensor_scalar(sc[0:1, 7:8], sc[0:1, 3:4], 1.0 / 128.0, None,
                            op0=mybir.AluOpType.mult)
    nc.vector.tensor_mul(sc[0:1, 7:8], sc[0:1, 7:8], sc[0:1, 8:9])

    # broadcast [thr, nthr, scale, r] to 128 partitions
    scb_ps = pp.tile([128, 4], F32, name="scb_ps", tag="prep")
    nc.tensor.matmul(scb_ps[:], ones_row[:], sc[0:1, 4:8], start=True, stop=True)
    nc.vector.tensor_copy(scal128[:, 0:4], scb_ps[:])
    thr128 = scal128[:, 0:1]
    nthr128 = scal128[:, 1:2]
    scale128 = scal128[:, 2:3]
    r128 = scal128[:, 3:4]

    # bias broadcast to 128 partitions
    for n in range(NCH):
        bch = bch_pool.tile([1, 512], F32, name=f"bch_{n}", tag="bch")
        nc.sync.dma_start(bch[:], b_ap[0:1, n * 512:(n + 1) * 512])
        b_ps = pp.tile([128, 512], F32, name=f"b_ps_{n}", tag="prep")
        nc.tensor.matmul(b_ps[:], ones_row[:], bch[:], start=True, stop=True)
        nc.vector.tensor_copy(bias_rep[:, n * 512:(n + 1) * 512], b_ps[:])
    psum_prep.__exit__(None, None, None)

    # -------- quantize the weight shard (already k-major: no transpose) ----
    for j in range(KT):
        w_h = io_pool.tile([128, 2048], F32, name=f"wh2_{j}", tag="io")
        nc.sync.dma_start(w_h[:, 0:n_sh], w_ap[j * 128:(j + 1) * 128, :])
        neg = neg_pool.tile([128, 2048], F32, name=f"neg_{j}", tag="neg")
        nc.gpsimd.tensor_scalar(neg[:, 0:n_sh], w_h[:, 0:n_sh], nthr128, None,
                                op0=mybir.AluOpType.is_lt)
        # wq = (w > thr) - (w < -thr)   in {-1, 0, 1}, bf16
        nc.vector.scalar_tensor_tensor(
            wqT[:, j * n_sh:(j + 1) * n_sh], w_h[:, 0:n_sh], thr128,
            neg[:, 0:n_sh],
            op0=mybir.AluOpType.is_gt, op1=mybir.AluOpType.subtract)

    po_pool = tc.tile_pool(name="po", bufs=8, space="PSUM")
    po = po_pool.__enter__()

    # ---------------- main loop over m-tiles ----------------
    for i in range(MT):
        m0 = i * 128
        xq_t = xq_pool.tile([128, KT * 128], BF16, name=f"xq_{i}", tag="xq")
        for g in range(NKG):
            xs = io_pool.tile([128, KG * 128], F32, name=f"xs_{i}_{g}", tag="io")
            src = xt_ap[g * KG * 128:(g + 1) * KG * 128, m0:m0 + 128]
            nc.sync.dma_start(
                xs[:].rearrange("p (kt mj) -> p kt mj", kt=KG),
                src.rearrange("(kt p) mj -> p kt mj", p=128))
            # xs = round_to_int(x*s), in place: magic-add rounds half-to-even
            nc.scalar.activation(xs[:], xs[:],
                                 mybir.ActivationFunctionType.Identity,
                                 bias=magic128[:], scale=scale128)
            # xq = min(xs - magic, 127) -> bf16 (>= -128 by construction)
            nc.vector.tensor_scalar(
                xq_t[:, g * KG * 128:(g + 1) * KG * 128], xs[:],
                MAGIC, 127.0,
                op0=mybir.AluOpType.subtract, op1=mybir.AluOpType.min)

        pous = [po.tile([128, 512], F32, name=f"po_{i}_{n}", tag="po")
                for n in range(NCH)]
        for k in range(KT):
            lhsT = xq_t[:, k * 128:(k + 1) * 128]
            for n in range(NCH):
                nc.tensor.matmul(
                    pous[n][:], lhsT,
                    wqT[:, k * n_sh + n * 512: k * n_sh + (n + 1) * 512],
                    start=(k == 0), stop=(k == KT - 1))

        for half in range(max(1, NCH // 2)):
            o_t = out_pool.tile([128, 1024], F32, name=f"ot_{i}_{half}",
                                tag="outp")
            w_out = min(1024, n_sh)
            for nn in range(min(2, NCH)):
                pidx = half * 2 + nn
                # out = psum * r + bias
                nc.vector.scalar_tensor_tensor(
                    o_t[:, nn * 512:(nn + 1) * 512], pous[pidx][:], r128,
                    bias_rep[:, pidx * 512:(pidx + 1) * 512],
                    op0=mybir.AluOpType.mult, op1=mybir.AluOpType.add)
            nc.sync.dma_start(
                o_ap[m0:m0 + 128, half * 1024:half * 1024 + w_out],
                o_t[:, 0:w_out])

    po_pool.__exit__(None, None, None)


def kernel(**inputs):
    x = np.ascontiguousarray(inputs["input"], dtype=np.float32).reshape(M, IN_F)
    w = np.ascontiguousarray(inputs["weight"], dtype=np.float32)
    b = np.ascontiguousarray(inputs["bias"], dtype=np.float32)

    xT = np.ascontiguousarray(x.T)            # [IN_F, M]
    wT = np.ascontiguousarray(w.T)            # [IN_F, OUT_F]

    if "nc" not in _CACHE:
        _CACHE["nc"] = _build()
    nc = _CACHE["nc"]

    in_maps = []
    for c in range(N_CORES):
        in_maps.append({
            "xt": xT,
            "x1": np.ascontiguousarray(xT[:, c * M_SL:(c + 1) * M_SL]),
            "w": np.ascontiguousarray(wT[:, c * N_SH:(c + 1) * N_SH]),
            "b": b[c * N_SH:(c + 1) * N_SH].reshape(1, N_SH),
        })
    res = bass_utils.run_bass_kernel_spmd(nc, in_maps,
                                          core_ids=list(range(N_CORES)))
    _CACHE["last_results"] = res
    _CACHE["last_in_maps"] = in_maps
    out = np.concatenate([r["o"] for r in res.results], axis=1)
    return out.reshape(B, S, OUT_F)


# revision 6
# speedup vs baseline: 1.0989x; 1.0558x over previous
"""BitLinear 1.58-bit (nn_BitLinear158) Trainium2 kernel, 8-core tensor-parallel.

Math (must match reference):
  gamma_x = max(max|x|, eps); s = 128/gamma_x; xq = clip(round(x*s), -128, 127)
  gamma_w = max(mean|w|, eps); wq = clip(round(w/gamma_w), -1, 1)  (ternary)
  out = (xq @ wq.T) * (gamma_w / s) + bias

Key facts exploited:
  - xq in [-128,127] and wq in {-1,0,1} are exact in bf16; products and all
    PSUM partial sums are integers < 2^20, exact in fp32 => the GEMM runs at
    full bf16 PE rate and is bit-identical to the fp32 reference einsum.
  - wq = 1[w > 0.5*gamma_w] - 1[w < -0.5*gamma_w] (no division / round).
  - round-half-even via the fp32 magic constant 1.5*2^23 (valid for |v|<=2^22).

v2 vs baseline:
  - x and w are handed over TRANSPOSED (k-major) by the host wrapper, so
    both GEMM operands arrive with the contraction dim on partitions and the
    PE never runs a transpose: it executes matmuls only.
  - Redundant InstLdweights (same stationary tile as the previous matmul)
    are rewritten to no-ops post-scheduling: 4 matmuls (n-chunks) share one
    weight load.
  - One AllGather replaces the two AllReduces for (sum|w|, max|x|).
  - All 8 PSUM banks double-buffer the accumulation groups.

Sharding: weight/bias split over out_features (16384 -> 8 x 2048), x
replicated; per-core GEMM [8192,4096]x[4096,2048].
"""

from contextlib import ExitStack

import numpy as np

import concourse.bass as bass
import concourse.mybir as mybir
import concourse.tile as tile
from concourse import bass_utils
from concourse.masks import make_identity
from concourse.vector_clock import ScopedClock

# ---------------------------------------------------------------------------
# Workaround: this walrus build rejects instructions carrying >1-2 sync wait
# commands. Tile's tail drain (emitted after tile_legalize) aggregates one
# wait per outstanding proc onto a single InstDrain and so escapes the
# wait-count legalization. Redistribute its waits across a chain of NO-queue
# nops (same sequencer => program order preserves the barrier semantics).
# ---------------------------------------------------------------------------
_MAX_WAITS = 1


def _patched_drain_and_barrier(self, tick_clock, wait_clock):
    nc = self.nc
    probe = nc.sync.nop()
    wait_clock.add_sem_waits(probe.ins, ScopedClock({None: tick_clock.global_clock}))
    si = probe.ins.sync_info
    waits = list(si.on_wait) if si is not None and si.on_wait else []
    ups = list(si.on_update) if si is not None and si.on_update else []
    probe.ins.sync_info = mybir.SyncInfo(on_wait=waits[:_MAX_WAITS], on_update=ups)
    rest = waits[_MAX_WAITS:]
    while rest:
        n2 = nc.sync.nop()
        n2.ins.sync_info = mybir.SyncInfo(on_wait=rest[:_MAX_WAITS], on_update=[])
        rest = rest[_MAX_WAITS:]

    nc.sync.drain()

    nc.all_engine_barrier()
    assert self.sems is not None
    popped = nc._tile_sem_poison_stack.pop()
    assert popped is self._sem_poison
    nc.clear_and_free_semaphores(list(self.sems.allocated().values()))
    nc.all_engine_barrier()


tile.TileContext._drain_and_barrier = _patched_drain_and_barrier

_nop_counter = [0]


def _legalize_waits(nc):
    """Split >_MAX_WAITS sync waits per instruction onto same-engine nops
    inserted immediately before (per-engine program order => semantics kept)."""
    for f in nc.m.functions:
        for blk in f.blocks:
            out = []
            changed = False
            for inst in blk.instructions:
                si = getattr(inst, "sync_info", None)
                waits = list(si.on_wait) if si is not None and si.on_wait else []
                if len(waits) > _MAX_WAITS and inst.engine != mybir.EngineType.Unassigned:
                    while len(waits) > _MAX_WAITS:
                        chunk, waits = waits[:_MAX_WAITS], waits[_MAX_WAITS:]
                        _nop_counter[0] += 1
                        out.append(mybir.InstNoOp(
                            name=f"waitnop-{_nop_counter[0]}",
                            engine=inst.engine, ins=[], outs=[],
                            sync_info=mybir.SyncInfo(on_wait=chunk, on_update=[]),
                        ))
                    inst.sync_info = mybir.SyncInfo(
                        on_wait=waits,
                        on_update=list(si.on_update) if si.on_update else [])
                    changed = True
                out.append(inst)
            if changed:
                blk.instructions = out


def _ldw_key(inst):
    ap = inst.ins[0]
    return (str(ap.ap), ap.offset, str(ap.dtype), ap.memref)


def _dedup_ldweights(nc, verbose=False):
    """Rewrite InstLdweights that reload the stationary tile already resident
    in the PE array into no-ops (PE weight regs persist across matmuls; only
    transpose-mode matmuls clobber them)."""
    total = dropped = 0
    for f in nc.m.functions:
        for blk in f.blocks:
            out = []
            last = None
            changed = False
            for inst in blk.instructions:
                if inst.engine == mybir.EngineType.PE:
                    tn = type(inst).__name__
                    if tn == "InstLdweights":
                        total += 1
                        key = _ldw_key(inst)
                        if key == last:
                            dropped += 1
                            changed = True
                            si = inst.sync_info
                            has_sync = si is not None and (si.on_wait or si.on_update)
                            if has_sync:
                                _nop_counter[0] += 1
                                out.append(mybir.InstNoOp(
                                    name=f"ldwnop-{_nop_counter[0]}",
                                    engine=mybir.EngineType.PE, ins=[], outs=[],
                                    sync_info=si))
                            continue
                        last = key
                    elif tn == "InstMatmult":
                        if getattr(inst, "is_transpose", False):
                            last = None
                    elif tn in ("InstNoOp", "InstEventSemaphore", "InstDrain",
                                "InstRegisterMove", "InstUnconditionalBranch"):
                        pass
                    else:
                        last = None
                out.append(inst)
            if changed:
                blk.instructions = out
    if verbose:
        print(f"_dedup_ldweights: dropped {dropped}/{total}")
    return dropped, total


# ---------------------------------------------------------------------------

N_CORES = 8
B, S, IN_F, OUT_F = 4, 2048, 4096, 16384
M = B * S                    # 8192 rows of x
N_SH = OUT_F // N_CORES      # 2048 output features per core
M_SL = M // N_CORES          # per-core slice of x for the pass-1 max
EPS = 1e-5
MAGIC = 12582912.0           # 1.5 * 2^23: fp32 round-to-nearest-even trick
F32 = mybir.dt.float32
BF16 = mybir.dt.bfloat16

_CACHE = {}


def _build(collective=True, m=M, in_f=IN_F, n_sh=N_SH, m_sl=M_SL, out_f=OUT_F,
           postpasses=True):
    nc = bass.Bass("TRN2", target_bir_lowering=False, debug=False,
                   num_devices=N_CORES if collective else 1)
    xt_ap = nc.dram_tensor("xt", [in_f, m], F32, kind="ExternalInput").ap()
    x1_ap = nc.dram_tensor("x1", [in_f, m_sl], F32, kind="ExternalInput").ap()
    w_ap = nc.dram_tensor("w", [in_f, n_sh], F32, kind="ExternalInput").ap()
    b_ap = nc.dram_tensor("b", [1, n_sh], F32, kind="ExternalInput").ap()
    o_ap = nc.dram_tensor("o", [m, n_sh], F32, kind="ExternalOutput").ap()

    with tile.TileContext(nc) as tc:
        with ExitStack() as stack:
            _body(nc, tc, stack, xt_ap, x1_ap, w_ap, b_ap, o_ap,
                  collective=collective, m=m, in_f=in_f, n_sh=n_sh,
                  m_sl=m_sl, out_f=out_f)
    if postpasses:
        _dedup_ldweights(nc, verbose=True)
        _legalize_waits(nc)
    return nc


def _body(nc, tc, stack, xt_ap, x1_ap, w_ap, b_ap, o_ap, collective,
          m, in_f, n_sh, m_sl, out_f):
    KT = in_f // 128             # k-tiles
    MT = m // 128                # m-tiles
    NCH = n_sh // 512            # psum column chunks per m-tile
    KG = min(16, KT)             # k-tiles per staging DMA
    NKG = KT // KG               # staging DMAs per m-tile

    def pool(name, bufs, space="SBUF"):
        return stack.enter_context(
            tc.tile_pool(name=name, bufs=bufs, space=space))

    # --- persistent SBUF ---
    wq_pool = pool("wq", 1)
    # wqT layout: [128 k-part, KT * n_sh] bf16, k-tile major
    wqT = wq_pool.tile([128, KT * n_sh], BF16, name="wqT", tag="wqT")
    const_pool = pool("const", 1)
    ident_f32 = const_pool.tile([128, 128], F32, name="ident_f32", tag="if32")
    ones_row = const_pool.tile([1, 128], F32, name="ones_row", tag="ones")
    bias_rep = const_pool.tile([128, n_sh], F32, name="bias_rep", tag="brep")
    scal128 = const_pool.tile([128, 4], F32, name="scal128", tag="scal128")
    magic128 = const_pool.tile([128, 1], F32, name="magic128", tag="magic")
    stats_pool = pool("stats", 1)
    wsums = stats_pool.tile([128, KT], F32, name="wsums", tag="wsums")
    xmaxs = stats_pool.tile([128, KT], F32, name="xmaxs", tag="xmaxs")
    stats2 = stats_pool.tile([128, 2], F32, name="stats2", tag="stats2")
    statsT_w = stats_pool.tile([1, 128], F32, name="statsT_w", tag="statsTw")
    statsT_x = stats_pool.tile([1, 128], F32, name="statsT_x", tag="statsTx")
    sc = stats_pool.tile([1, 12], F32, name="sc", tag="sc")
    ag = stats_pool.tile([1, 2 * N_CORES], F32, name="ag", tag="ag")

    # --- rotating SBUF ---
    io_pool = pool("io", 3)          # [128, 2048] f32 staging (w / x slabs)
    neg_pool = pool("neg", 2)        # [128, 2048] f32 scratch for w quantize
    xq_pool = pool("xq", 2)          # [128, KT*128] bf16 quantized m-slab
    out_pool = pool("outp", 2)       # [128, 1024] f32 output staging
    bch_pool = pool("bch", 2)        # [1, 512] f32 bias chunks

    make_identity(nc, ident_f32[:])
    nc.gpsimd.memset(ones_row[:], 1.0)
    nc.gpsimd.memset(magic128[:], MAGIC)

    psum_prep = tc.tile_pool(name="psum_prep", bufs=2, space="PSUM")
    pp = psum_prep.__enter__()

    # ---------------- pass 1: |w| row sums + sliced max|x| ----------------
    # x responsibility for the global max is M-sharded across cores (each
    # core scans 1/8 of x = its x1 input, columns of xT); an AllReduce(max)
    # recovers the exact global max.
    for j in range(KT):
        w_h = io_pool.tile([128, 2048], F32, name=f"wh_{j}", tag="io")
        nc.sync.dma_start(w_h[:, 0:n_sh], w_ap[j * 128:(j + 1) * 128, :])
        nc.vector.tensor_reduce(wsums[:, j:j + 1], w_h[:, 0:n_sh],
                                axis=mybir.AxisListType.X,
                                op=mybir.AluOpType.add,
                                apply_absolute_value=True)
    nc.vector.tensor_reduce(stats2[:, 0:1], wsums[:],
                            axis=mybir.AxisListType.X, op=mybir.AluOpType.add)

    for j in range(KT // 2):
        x_h = io_pool.tile([128, 2048], F32, name=f"xh1_{j}", tag="io")
        nc.sync.dma_start(x_h[:, 0:m_sl],
                          x1_ap[(2 * j) * 128:(2 * j + 1) * 128, :])
        nc.sync.dma_start(x_h[:, m_sl:2 * m_sl],
                          x1_ap[(2 * j + 1) * 128:(2 * j + 2) * 128, :])
        nc.vector.tensor_reduce(xmaxs[:, j:j + 1], x_h[:, 0:2 * m_sl],
                                axis=mybir.AxisListType.X,
                                op=mybir.AluOpType.max,
                                apply_absolute_value=True)
    nc.vector.tensor_reduce(stats2[:, 1:2], xmaxs[:, 0:KT // 2],
                            axis=mybir.AxisListType.X, op=mybir.AluOpType.max)

    # cross-partition reductions via PE transpose
    st_ps_w = pp.tile([1, 128], F32, name="st_ps_w", tag="prep")
    nc.tensor.transpose(st_ps_w[:], stats2[:, 0:1], ident_f32[:])
    nc.vector.tensor_copy(statsT_w[:], st_ps_w[:])
    nc.vector.tensor_reduce(sc[0:1, 0:1], statsT_w[:],
                            axis=mybir.AxisListType.X, op=mybir.AluOpType.add)
    st_ps_x = pp.tile([1, 128], F32, name="st_ps_x", tag="prep")
    nc.tensor.transpose(st_ps_x[:], stats2[:, 1:2], ident_f32[:])
    nc.vector.tensor_copy(statsT_x[:], st_ps_x[:])
    nc.vector.tensor_reduce(sc[0:1, 1:2], statsT_x[:],
                            axis=mybir.AxisListType.X, op=mybir.AluOpType.max)

    if collective:
        dram_pool = pool("dram", 1, space="DRAM")
        cc_in = dram_pool.tile([1, 2], F32, name="cc_in", tag="cc_in")
        cc_out = dram_pool.tile([1, 2 * N_CORES], F32, name="cc_out",
                                tag="cc_out", addr_space="Shared")
        nc.gpsimd.dma_start(cc_in[:], sc[0:1, 0:2])
        nc.gpsimd.collective_compute(
            "AllGather", mybir.AluOpType.bypass,
            replica_groups=[list(range(N_CORES))],
            ins=[cc_in[:].opt()], outs=[cc_out[:].opt()],
        )
        nc.gpsimd.dma_start(ag[:], cc_out[:])
        # core-major [w0, x0, w1, x1, ...] -> strided views
        ag3 = ag[:].rearrange("p (c t) -> p t c", t=2)
        nc.vector.tensor_reduce(sc[0:1, 2:3], ag3[0:1, 0:1, :],
                                axis=mybir.AxisListType.X,
                                op=mybir.AluOpType.add)
        nc.vector.tensor_reduce(sc[0:1, 3:4], ag3[0:1, 1:2, :],
                                axis=mybir.AxisListType.X,
                                op=mybir.AluOpType.max)
        wsum_all = sc[0:1, 2:3]
        xmax_all = sc[0:1, 3:4]
        inv_cnt = 1.0 / (out_f * in_f)
    else:  # single-core sim variant: local stats stand in for global ones
        wsum_all = sc[0:1, 0:1]
        xmax_all = sc[0:1, 1:2]
        inv_cnt = 1.0 / (n_sh * in_f)

    # gamma_w = max(sum/count, eps) -> sc[0,8]
    nc.vector.tensor_scalar(sc[0:1, 8:9], wsum_all, inv_cnt, EPS,
                            op0=mybir.AluOpType.mult, op1=mybir.AluOpType.max)
    # thr = 0.5*gamma_w -> sc[0,4]; nthr -> sc[0,5]
    nc.vector.tensor_scalar(sc[0:1, 4:5], sc[0:1, 8:9], 0.5, None,
                            op0=mybir.AluOpType.mult)
    nc.vector.tensor_scalar(sc[0:1, 5:6], sc[0:1, 8:9], -0.5, None,
                            op0=mybir.AluOpType.mult)
    # gamma_x = max(xmax, eps) -> sc[0,3] slot
    nc.vector.tensor_scalar(sc[0:1, 3:4], xmax_all, EPS, None,
                            op0=mybir.AluOpType.max)
    # scale_x = 128 / gamma_x -> sc[0,6]
    nc.vector.reciprocal(sc[0:1, 6:7], sc[0:1, 3:4])
    nc.vector.tensor_scalar(sc[0:1, 6:7], sc[0:1, 6:7], 128.0, None,
                            op0=mybir.AluOpType.mult)
    # r = gamma_w * gamma_x / 128 -> sc[0,7]
    nc.vector.tensor_scalar(sc[0:1, 7:8], sc[0:1, 3:4], 1.0 / 128.0, None,
                            op0=mybir.AluOpType.mult)
    nc.vector.tensor_mul(sc[0:1, 7:8], sc[0:1, 7:8], sc[0:1, 8:9])

    # broadcast [thr, nthr, scale, r] to 128 partitions
    scb_ps = pp.tile([128, 4], F32, name="scb_ps", tag="prep")
    nc.tensor.matmul(scb_ps[:], ones_row[:], sc[0:1, 4:8], start=True, stop=True)
    nc.vector.tensor_copy(scal128[:, 0:4], scb_ps[:])
    thr128 = scal128[:, 0:1]
    nthr128 = scal128[:, 1:2]
    scale128 = scal128[:, 2:3]
    r128 = scal128[:, 3:4]

    # bias broadcast to 128 partitions
    for n in range(NCH):
        bch = bch_pool.tile([1, 512], F32, name=f"bch_{n}", tag="bch")
        nc.sync.dma_start(bch[:], b_ap[0:1, n * 512:(n + 1) * 512])
        b_ps = pp.tile([128, 512], F32, name=f"b_ps_{n}", tag="prep")
        nc.tensor.matmul(b_ps[:], ones_row[:], bch[:], start=True, stop=True)
        nc.vector.tensor_copy(bias_rep[:, n * 512:(n + 1) * 512], b_ps[:])
    psum_prep.__exit__(None, None, None)

    # -------- quantize the weight shard (already k-major: no transpose) ----
    for j in range(KT):
        w_h = io_pool.tile([128, 2048], F32, name=f"wh2_{j}", tag="io")
        nc.sync.dma_start(w_h[:, 0:n_sh], w_ap[j * 128:(j + 1) * 128, :])
        neg = neg_pool.tile([128, 2048], F32, name=f"neg_{j}", tag="neg")
        nc.gpsimd.tensor_scalar(neg[:, 0:n_sh], w_h[:, 0:n_sh], nthr128, None,
                                op0=mybir.AluOpType.is_lt)
        # wq = (w > thr) - (w < -thr)   in {-1, 0, 1}, bf16
        nc.vector.scalar_tensor_tensor(
            wqT[:, j * n_sh:(j + 1) * n_sh], w_h[:, 0:n_sh], thr128,
            neg[:, 0:n_sh],
            op0=mybir.AluOpType.is_gt, op1=mybir.AluOpType.subtract)

    po_pool = tc.tile_pool(name="po", bufs=8, space="PSUM")
    po = po_pool.__enter__()

    # ---------------- main loop over m-tiles ----------------
    # The drain of m-tile i-1 is emitted AFTER the quantize of m-tile i:
    # the drain stt WAITS on the matmuls, and the DVE queue is strict FIFO,
    # so it must sit behind the quantize work feeding the next matmul group
    # or it head-of-line-blocks the whole pipeline. Output DMAs go on the
    # otherwise-idle gpsimd queue for the same reason (they wait on the
    # drain; SP must keep streaming x loads).
    def drain(pi, ppous):
        pm0 = pi * 128
        for half in range(max(1, NCH // 2)):
            o_t = out_pool.tile([128, 1024], F32, name=f"ot_{pi}_{half}",
                                tag="outp")
            w_out = min(1024, n_sh)
            for nn in range(min(2, NCH)):
                pidx = half * 2 + nn
                # out = psum * r + bias
                nc.vector.scalar_tensor_tensor(
                    o_t[:, nn * 512:(nn + 1) * 512], ppous[pidx][:], r128,
                    bias_rep[:, pidx * 512:(pidx + 1) * 512],
                    op0=mybir.AluOpType.mult, op1=mybir.AluOpType.add)
            nc.gpsimd.dma_start(
                o_ap[pm0:pm0 + 128, half * 1024:half * 1024 + w_out],
                o_t[:, 0:w_out])

    prev = None
    for i in range(MT):
        m0 = i * 128
        xq_t = xq_pool.tile([128, KT * 128], BF16, name=f"xq_{i}", tag="xq")
        for g in range(NKG):
            xs = io_pool.tile([128, KG * 128], F32, name=f"xs_{i}_{g}", tag="io")
            src = xt_ap[g * KG * 128:(g + 1) * KG * 128, m0:m0 + 128]
            nc.sync.dma_start(
                xs[:].rearrange("p (kt mj) -> p kt mj", kt=KG),
                src.rearrange("(kt p) mj -> p kt mj", p=128))
            # xs = round_to_int(x*s), in place: magic-add rounds half-to-even
            nc.scalar.activation(xs[:], xs[:],
                                 mybir.ActivationFunctionType.Identity,
                                 bias=magic128[:], scale=scale128)
            # xq = min(xs - magic, 127) -> bf16 (>= -128 by construction)
            nc.vector.tensor_scalar(
                xq_t[:, g * KG * 128:(g + 1) * KG * 128], xs[:],
                MAGIC, 127.0,
                op0=mybir.AluOpType.subtract, op1=mybir.AluOpType.min)

        if prev is not None:
            drain(*prev)

        pous = [po.tile([128, 512], F32, name=f"po_{i}_{n}", tag="po")
                for n in range(NCH)]
        for k in range(KT):
            lhsT = xq_t[:, k * 128:(k + 1) * 128]
            for n in range(NCH):
                nc.tensor.matmul(
                    pous[n][:], lhsT,
                    wqT[:, k * n_sh + n * 512: k * n_sh + (n + 1) * 512],
                    start=(k == 0), stop=(k == KT - 1))
        prev = (i, pous)

    drain(*prev)
    po_pool.__exit__(None, None, None)


def kernel(**inputs):
    x = np.ascontiguousarray(inputs["input"], dtype=np.float32).reshape(M, IN_F)
    w = np.ascontiguousarray(inputs["weight"], dtype=np.float32)
    b = np.ascontiguousarray(inputs["bias"], dtype=np.float32)

    xT = np.ascontiguousarray(x.T)            # [IN_F, M]
    wT = np.ascontiguousarray(w.T)            # [IN_F, OUT_F]

    if "nc" not in _CACHE:
        _CACHE["nc"] = _build()
    nc = _CACHE["nc"]

    in_maps = []
    for c in range(N_CORES):
        in_maps.append({
            "xt": xT,
            "x1": np.ascontiguousarray(xT[:, c * M_SL:(c + 1) * M_SL]),
            "w": np.ascontiguousarray(wT[:, c * N_SH:(c + 1) * N_SH]),
            "b": b[c * N_SH:(c + 1) * N_SH].reshape(1, N_SH),
        })
    res = bass_utils.run_bass_kernel_spmd(nc, in_maps,
                                          core_ids=list(range(N_CORES)))
    _CACHE["last_results"] = res
    _CACHE["last_in_maps"] = in_maps
    out = np.concatenate([r["o"] for r in res.results], axis=1)
    return out.reshape(B, S, OUT_F)


# revision 14
# speedup vs baseline: 1.1679x; 1.0628x over previous
"""BitLinear 1.58-bit (nn_BitLinear158) Trainium2 kernel, 8-core tensor-parallel.

Math (must match reference):
  gamma_x = max(max|x|, eps); s = 128/gamma_x; xq = clip(round(x*s), -128, 127)
  gamma_w = max(mean|w|, eps); wq = clip(round(w/gamma_w), -1, 1)  (ternary)
  out = (xq @ wq.T) * (gamma_w / s) + bias

Key facts exploited:
  - xq in [-128,127] and wq in {-1,0,1} are exact in bf16; products and all
    PSUM partial sums are integers < 2^20, exact in fp32 => the GEMM runs at
    full bf16 PE rate and is bit-identical to the fp32 reference einsum.
  - wq = 1[w > 0.5*gamma_w] - 1[w < -0.5*gamma_w] (no division / round).
  - round-half-even via the fp32 magic constant 1.5*2^23 (valid for |v|<=2^22).

v2 vs baseline:
  - x and w are handed over TRANSPOSED (k-major) by the host wrapper, so
    both GEMM operands arrive with the contraction dim on partitions and the
    PE never runs a transpose: it executes matmuls only.
  - Redundant InstLdweights (same stationary tile as the previous matmul)
    are rewritten to no-ops post-scheduling: 4 matmuls (n-chunks) share one
    weight load.
  - One AllGather replaces the two AllReduces for (sum|w|, max|x|).
  - All 8 PSUM banks double-buffer the accumulation groups.

Sharding: weight/bias split over out_features (16384 -> 8 x 2048), x
replicated; per-core GEMM [8192,4096]x[4096,2048].
"""

from contextlib import ExitStack

import numpy as np

import concourse.bass as bass
import concourse.mybir as mybir
import concourse.tile as tile
from concourse import bass_utils
from concourse.masks import make_identity
from concourse.vector_clock import ScopedClock

# ---------------------------------------------------------------------------
# Workaround: this walrus build rejects instructions carrying >1-2 sync wait
# commands. Tile's tail drain (emitted after tile_legalize) aggregates one
# wait per outstanding proc onto a single InstDrain and so escapes the
# wait-count legalization. Redistribute its waits across a chain of NO-queue
# nops (same sequencer => program order preserves the barrier semantics).
# ---------------------------------------------------------------------------
_MAX_WAITS = 1


def _patched_drain_and_barrier(self, tick_clock, wait_clock):
    nc = self.nc
    probe = nc.sync.nop()
    wait_clock.add_sem_waits(probe.ins, ScopedClock({None: tick_clock.global_clock}))
    si = probe.ins.sync_info
    waits = list(si.on_wait) if si is not None and si.on_wait else []
    ups = list(si.on_update) if si is not None and si.on_update else []
    probe.ins.sync_info = mybir.SyncInfo(on_wait=waits[:_MAX_WAITS], on_update=ups)
    rest = waits[_MAX_WAITS:]
    while rest:
        n2 = nc.sync.nop()
        n2.ins.sync_info = mybir.SyncInfo(on_wait=rest[:_MAX_WAITS], on_update=[])
        rest = rest[_MAX_WAITS:]

    nc.sync.drain()

    nc.all_engine_barrier()
    assert self.sems is not None
    popped = nc._tile_sem_poison_stack.pop()
    assert popped is self._sem_poison
    nc.clear_and_free_semaphores(list(self.sems.allocated().values()))
    nc.all_engine_barrier()


tile.TileContext._drain_and_barrier = _patched_drain_and_barrier

_nop_counter = [0]


def _legalize_waits(nc):
    """Split >_MAX_WAITS sync waits per instruction onto same-engine nops
    inserted immediately before (per-engine program order => semantics kept)."""
    for f in nc.m.functions:
        for blk in f.blocks:
            out = []
            changed = False
            for inst in blk.instructions:
                si = getattr(inst, "sync_info", None)
                waits = list(si.on_wait) if si is not None and si.on_wait else []
                if len(waits) > _MAX_WAITS and inst.engine != mybir.EngineType.Unassigned:
                    while len(waits) > _MAX_WAITS:
                        chunk, waits = waits[:_MAX_WAITS], waits[_MAX_WAITS:]
                        _nop_counter[0] += 1
                        out.append(mybir.InstNoOp(
                            name=f"waitnop-{_nop_counter[0]}",
                            engine=inst.engine, ins=[], outs=[],
                            sync_info=mybir.SyncInfo(on_wait=chunk, on_update=[]),
                        ))
                    inst.sync_info = mybir.SyncInfo(
                        on_wait=waits,
                        on_update=list(si.on_update) if si.on_update else [])
                    changed = True
                out.append(inst)
            if changed:
                blk.instructions = out


def _strip_mm_incs(nc, verbose=False):
    """Remove the vector-clock sem-inc from non-group-final matmuls (their
    only purpose is advancing the PE clock; consumers wait on group-end
    counts). All waits on that semaphore are remapped onto the kept-inc
    numbering, rounding up so no dependency is ever released early."""
    import collections

    for f in nc.m.functions:
        for blk in f.blocks:
            pe_insts = [i for i in blk.instructions
                        if i.engine == mybir.EngineType.PE]
            cnt = collections.Counter()
            for i in pe_insts:
                si = getattr(i, "sync_info", None)
                for u in (si.on_update if si is not None and si.on_update else []):
                    if u.update_mode == "sem-inc":
                        cnt[u.id] += 1
            if not cnt or cnt.most_common(1)[0][1] < 100:
                continue
            sem_id = cnt.most_common(1)[0][0]

            incs = []  # (inst, strippable)
            for i in pe_insts:
                si = getattr(i, "sync_info", None)
                ups = [u for u in (si.on_update if si is not None and si.on_update else [])
                       if u.id == sem_id]
                if not ups:
                    continue
                assert len(ups) == 1 and ups[0].update_value == 1
                strippable = (type(i).__name__ == "InstMatmult"
                              and getattr(i, "stop_tensor_calc", True) is False)
                incs.append([i, strippable])
            if incs:
                incs[-1][1] = False  # final inc must survive (drain waits total)

            kept_prefix = []
            k = 0
            for _, s in incs:
                if not s:
                    k += 1
                kept_prefix.append(k)
            kept_total = k

            def remap(v):
                if v <= 0:
                    return v
                vi = min(v, len(incs))
                nv = kept_prefix[vi - 1]
                if incs[vi - 1][1]:  # waited-on inc was stripped: round up
                    nv += 1
                assert nv <= kept_total
                return nv

            # rewrite waits everywhere, then strip updates
            nwait = nstrip = 0
            for i in blk.instructions:
                si = getattr(i, "sync_info", None)
                if si is None or not si.on_wait:
                    continue
                changed = False
                for w in si.on_wait:
                    if w.id == sem_id:
                        assert w.wait_mode == "sem-ge-imm", w.wait_mode
                        w.wait_value = remap(w.wait_value)
                        changed = True
                if changed:
                    nwait += 1
                    i.sync_info = mybir.SyncInfo(
                        on_wait=list(si.on_wait),
                        on_update=list(si.on_update) if si.on_update else [])
            for i, strippable in incs:
                if strippable:
                    si = i.sync_info
                    i.sync_info = mybir.SyncInfo(
                        on_wait=list(si.on_wait) if si.on_wait else [],
                        on_update=[u for u in si.on_update if u.id != sem_id])
                    nstrip += 1
            if verbose:
                print(f"_strip_mm_incs: stripped {nstrip} incs, "
                      f"remapped {nwait} waiters, kept {kept_total}")


def _ldw_key(inst):
    ap = inst.ins[0]
    return (str(ap.ap), ap.offset, str(ap.dtype), ap.memref)


def _dedup_ldweights(nc, verbose=False):
    """Rewrite InstLdweights that reload the stationary tile already resident
    in the PE array into no-ops (PE weight regs persist across matmuls; only
    transpose-mode matmuls clobber them)."""
    total = dropped = 0
    for f in nc.m.functions:
        for blk in f.blocks:
            out = []
            last = None
            changed = False
            for inst in blk.instructions:
                if inst.engine == mybir.EngineType.PE:
                    tn = type(inst).__name__
                    if tn == "InstLdweights":
                        total += 1
                        key = _ldw_key(inst)
                        if key == last:
                            dropped += 1
                            changed = True
                            si = inst.sync_info
                            has_sync = si is not None and (si.on_wait or si.on_update)
                            if has_sync:
                                _nop_counter[0] += 1
                                out.append(mybir.InstNoOp(
                                    name=f"ldwnop-{_nop_counter[0]}",
                                    engine=mybir.EngineType.PE, ins=[], outs=[],
                                    sync_info=si))
                            continue
                        last = key
                    elif tn == "InstMatmult":
                        if getattr(inst, "is_transpose", False):
                            last = None
                    elif tn in ("InstNoOp", "InstEventSemaphore", "InstDrain",
                                "InstRegisterMove", "InstUnconditionalBranch"):
                        pass
                    else:
                        last = None
                out.append(inst)
            if changed:
                blk.instructions = out
    if verbose:
        print(f"_dedup_ldweights: dropped {dropped}/{total}")
    return dropped, total


# ---------------------------------------------------------------------------

N_CORES = 8
B, S, IN_F, OUT_F = 4, 2048, 4096, 16384
M = B * S                    # 8192 rows of x
N_SH = OUT_F // N_CORES      # 2048 output features per core
M_SL = M // N_CORES          # per-core slice of x for the pass-1 max
EPS = 1e-5
MAGIC = 12582912.0           # 1.5 * 2^23: fp32 round-to-nearest-even trick
F32 = mybir.dt.float32
BF16 = mybir.dt.bfloat16

_CACHE = {}


def _build(collective=True, m=M, in_f=IN_F, n_sh=N_SH, m_sl=M_SL, out_f=OUT_F,
           postpasses=True, ablate=None):
    nc = bass.Bass("TRN2", target_bir_lowering=False, debug=False,
                   num_devices=N_CORES if collective else 1)
    xt_ap = nc.dram_tensor("xt", [in_f, m], F32, kind="ExternalInput").ap()
    x1_ap = nc.dram_tensor("x1", [in_f, m_sl], F32, kind="ExternalInput").ap()
    w_ap = nc.dram_tensor("w", [in_f, n_sh], F32, kind="ExternalInput").ap()
    b_ap = nc.dram_tensor("b", [1, n_sh], F32, kind="ExternalInput").ap()
    o_ap = nc.dram_tensor("o", [m, n_sh], F32, kind="ExternalOutput").ap()

    with tile.TileContext(nc) as tc:
        with ExitStack() as stack:
            _body(nc, tc, stack, xt_ap, x1_ap, w_ap, b_ap, o_ap,
                  collective=collective, m=m, in_f=in_f, n_sh=n_sh,
                  m_sl=m_sl, out_f=out_f, ablate=ablate)
    if postpasses:
        _dedup_ldweights(nc, verbose=True)
        _legalize_waits(nc)
    return nc


def _body(nc, tc, stack, xt_ap, x1_ap, w_ap, b_ap, o_ap, collective,
          m, in_f, n_sh, m_sl, out_f, ablate=None):
    KT = in_f // 128             # k-tiles
    MT = m // 128                # m-tiles
    NCH = n_sh // 512            # psum column chunks per m-tile
    KG = min(16, KT)             # k-tiles per staging DMA
    NKG = KT // KG               # staging DMAs per m-tile

    def pool(name, bufs, space="SBUF"):
        return stack.enter_context(
            tc.tile_pool(name=name, bufs=bufs, space=space))

    # --- persistent SBUF ---
    wq_pool = pool("wq", 1)
    # wqT layout: [128 k-part, KT * n_sh] bf16, k-tile major
    wqT = wq_pool.tile([128, KT * n_sh], BF16, name="wqT", tag="wqT")
    const_pool = pool("const", 1)
    ident_f32 = const_pool.tile([128, 128], F32, name="ident_f32", tag="if32")
    ones_row = const_pool.tile([1, 128], F32, name="ones_row", tag="ones")
    bias_rep = const_pool.tile([128, n_sh], F32, name="bias_rep", tag="brep")
    scal128 = const_pool.tile([128, 4], F32, name="scal128", tag="scal128")
    magic128 = const_pool.tile([128, 1], F32, name="magic128", tag="magic")
    stats_pool = pool("stats", 1)
    wsums = stats_pool.tile([128, KT], F32, name="wsums", tag="wsums")
    xmaxs = stats_pool.tile([128, KT], F32, name="xmaxs", tag="xmaxs")
    stats2 = stats_pool.tile([128, 2], F32, name="stats2", tag="stats2")
    statsT_w = stats_pool.tile([1, 128], F32, name="statsT_w", tag="statsTw")
    statsT_x = stats_pool.tile([1, 128], F32, name="statsT_x", tag="statsTx")
    sc = stats_pool.tile([1, 12], F32, name="sc", tag="sc")
    ag = stats_pool.tile([1, 2 * N_CORES], F32, name="ag", tag="ag")

    # --- rotating SBUF ---
    io_pool = pool("io", 3)          # [128, 2048] f32 staging (w / x slabs)
    neg_pool = pool("neg", 2)        # [128, 2048] f32 scratch for w quantize
    xq_pool = pool("xq", 2)          # [128, KT*128] bf16 quantized m-slab
    out_pool = pool("outp", 2)       # [128, 1024] f32 output staging
    bch_pool = pool("bch", 2)        # [1, 512] f32 bias chunks

    make_identity(nc, ident_f32[:])
    nc.gpsimd.memset(ones_row[:], 1.0)
    nc.gpsimd.memset(magic128[:], MAGIC)

    psum_prep = tc.tile_pool(name="psum_prep", bufs=2, space="PSUM")
    pp = psum_prep.__enter__()

    # ---------------- pass 1: |w| row sums + sliced max|x| ----------------
    # x responsibility for the global max is M-sharded across cores (each
    # core scans 1/8 of x = its x1 input, columns of xT); an AllReduce(max)
    # recovers the exact global max.
    for j in range(KT):
        w_h = io_pool.tile([128, 2048], F32, name=f"wh_{j}", tag="io")
        nc.sync.dma_start(w_h[:, 0:n_sh], w_ap[j * 128:(j + 1) * 128, :])
        # |w| row-sum on the (otherwise idle) scalar engine: out=|w| is a
        # throwaway in-place write, accum_out catches the per-partition sum.
        nc.scalar.activation(w_h[:, 0:n_sh], w_h[:, 0:n_sh],
                             mybir.ActivationFunctionType.Abs,
                             accum_out=wsums[:, j:j + 1])
    nc.vector.tensor_reduce(stats2[:, 0:1], wsums[:],
                            axis=mybir.AxisListType.X, op=mybir.AluOpType.add)

    for j in range(KT // 2):
        x_h = io_pool.tile([128, 2048], F32, name=f"xh1_{j}", tag="io")
        nc.sync.dma_start(x_h[:, 0:m_sl],
                          x1_ap[(2 * j) * 128:(2 * j + 1) * 128, :])
        nc.sync.dma_start(x_h[:, m_sl:2 * m_sl],
                          x1_ap[(2 * j + 1) * 128:(2 * j + 2) * 128, :])
        nc.vector.tensor_reduce(xmaxs[:, j:j + 1], x_h[:, 0:2 * m_sl],
                                axis=mybir.AxisListType.X,
                                op=mybir.AluOpType.max,
                                apply_absolute_value=True)
    nc.vector.tensor_reduce(stats2[:, 1:2], xmaxs[:, 0:KT // 2],
                            axis=mybir.AxisListType.X, op=mybir.AluOpType.max)

    # cross-partition reductions via PE transpose
    st_ps_w = pp.tile([1, 128], F32, name="st_ps_w", tag="prep")
    nc.tensor.transpose(st_ps_w[:], stats2[:, 0:1], ident_f32[:])
    nc.vector.tensor_copy(statsT_w[:], st_ps_w[:])
    nc.vector.tensor_reduce(sc[0:1, 0:1], statsT_w[:],
                            axis=mybir.AxisListType.X, op=mybir.AluOpType.add)
    st_ps_x = pp.tile([1, 128], F32, name="st_ps_x", tag="prep")
    nc.tensor.transpose(st_ps_x[:], stats2[:, 1:2], ident_f32[:])
    nc.vector.tensor_copy(statsT_x[:], st_ps_x[:])
    nc.vector.tensor_reduce(sc[0:1, 1:2], statsT_x[:],
                            axis=mybir.AxisListType.X, op=mybir.AluOpType.max)

    if collective:
        dram_pool = pool("dram", 1, space="DRAM")
        cc_in = dram_pool.tile([1, 2], F32, name="cc_in", tag="cc_in")
        cc_out = dram_pool.tile([1, 2 * N_CORES], F32, name="cc_out",
                                tag="cc_out", addr_space="Shared")
        nc.gpsimd.dma_start(cc_in[:], sc[0:1, 0:2])
        nc.gpsimd.collective_compute(
            "AllGather", mybir.AluOpType.bypass,
            replica_groups=[list(range(N_CORES))],
            ins=[cc_in[:].opt()], outs=[cc_out[:].opt()],
        )
        nc.gpsimd.dma_start(ag[:], cc_out[:])
        # core-major [w0, x0, w1, x1, ...] -> strided views
        ag3 = ag[:].rearrange("p (c t) -> p t c", t=2)
        nc.vector.tensor_reduce(sc[0:1, 2:3], ag3[0:1, 0:1, :],
                                axis=mybir.AxisListType.X,
                                op=mybir.AluOpType.add)
        nc.vector.tensor_reduce(sc[0:1, 3:4], ag3[0:1, 1:2, :],
                                axis=mybir.AxisListType.X,
                                op=mybir.AluOpType.max)
        wsum_all = sc[0:1, 2:3]
        xmax_all = sc[0:1, 3:4]
        inv_cnt = 1.0 / (out_f * in_f)
    else:  # single-core sim variant: local stats stand in for global ones
        wsum_all = sc[0:1, 0:1]
        xmax_all = sc[0:1, 1:2]
        inv_cnt = 1.0 / (n_sh * in_f)

    # gamma_w = max(sum/count, eps) -> sc[0,8]
    nc.vector.tensor_scalar(sc[0:1, 8:9], wsum_all, inv_cnt, EPS,
                            op0=mybir.AluOpType.mult, op1=mybir.AluOpType.max)
    # thr = 0.5*gamma_w -> sc[0,4]; nthr -> sc[0,5]
    nc.vector.tensor_scalar(sc[0:1, 4:5], sc[0:1, 8:9], 0.5, None,
                            op0=mybir.AluOpType.mult)
    nc.vector.tensor_scalar(sc[0:1, 5:6], sc[0:1, 8:9], -0.5, None,
                            op0=mybir.AluOpType.mult)
    # gamma_x = max(xmax, eps) -> sc[0,3] slot
    nc.vector.tensor_scalar(sc[0:1, 3:4], xmax_all, EPS, None,
                            op0=mybir.AluOpType.max)
    # scale_x = 128 / gamma_x -> sc[0,6]
    nc.vector.reciprocal(sc[0:1, 6:7], sc[0:1, 3:4])
    nc.vector.tensor_scalar(sc[0:1, 6:7], sc[0:1, 6:7], 128.0, None,
                            op0=mybir.AluOpType.mult)
    # r = gamma_w * gamma_x / 128 -> sc[0,7]
    nc.vector.tensor_scalar(sc[0:1, 7:8], sc[0:1, 3:4], 1.0 / 128.0, None,
                            op0=mybir.AluOpType.mult)
    nc.vector.tensor_mul(sc[0:1, 7:8], sc[0:1, 7:8], sc[0:1, 8:9])

    # broadcast [thr, nthr, scale, r] to 128 partitions
    scb_ps = pp.tile([128, 4], F32, name="scb_ps", tag="prep")
    nc.tensor.matmul(scb_ps[:], ones_row[:], sc[0:1, 4:8], start=True, stop=True)
    nc.vector.tensor_copy(scal128[:, 0:4], scb_ps[:])
    thr128 = scal128[:, 0:1]
    nthr128 = scal128[:, 1:2]
    scale128 = scal128[:, 2:3]
    r128 = scal128[:, 3:4]

    # bias broadcast to 128 partitions
    for n in range(NCH):
        bch = bch_pool.tile([1, 512], F32, name=f"bch_{n}", tag="bch")
        nc.sync.dma_start(bch[:], b_ap[0:1, n * 512:(n + 1) * 512])
        b_ps = pp.tile([128, 512], F32, name=f"b_ps_{n}", tag="prep")
        nc.tensor.matmul(b_ps[:], ones_row[:], bch[:], start=True, stop=True)
        nc.vector.tensor_copy(bias_rep[:, n * 512:(n + 1) * 512], b_ps[:])
    psum_prep.__exit__(None, None, None)

    if ablate == "pass1_only":
        return
    # -------- quantize the weight shard (already k-major: no transpose) ----
    for j in range(KT):
        w_h = io_pool.tile([128, 2048], F32, name=f"wh2_{j}", tag="io")
        nc.sync.dma_start(w_h[:, 0:n_sh], w_ap[j * 128:(j + 1) * 128, :])
        neg = neg_pool.tile([128, 2048], F32, name=f"neg_{j}", tag="neg")
        nc.vector.tensor_scalar(neg[:, 0:n_sh], w_h[:, 0:n_sh], nthr128, None,
                                op0=mybir.AluOpType.is_lt)
        # wq = (w > thr) - (w < -thr)   in {-1, 0, 1}, bf16
        nc.vector.scalar_tensor_tensor(
            wqT[:, j * n_sh:(j + 1) * n_sh], w_h[:, 0:n_sh], thr128,
            neg[:, 0:n_sh],
            op0=mybir.AluOpType.is_gt, op1=mybir.AluOpType.subtract)

    if ablate == "prep_only":
        return

    po_pool = tc.tile_pool(name="po", bufs=8, space="PSUM")
    po = po_pool.__enter__()

    # ---------------- main loop over m-tiles ----------------
    # The drain of m-tile i-1 is emitted AFTER the quantize of m-tile i:
    # the drain stt WAITS on the matmuls, and the DVE queue is strict FIFO,
    # so it must sit behind the quantize work feeding the next matmul group
    # or it head-of-line-blocks the whole pipeline. Output DMAs go on the
    # otherwise-idle gpsimd queue for the same reason (they wait on the
    # drain; SP must keep streaming x loads).
    def drain(pi, ppous):
        pm0 = pi * 128
        for half in range(max(1, NCH // 2)):
            o_t = out_pool.tile([128, 1024], F32, name=f"ot_{pi}_{half}",
                                tag="outp")
            w_out = min(1024, n_sh)
            for nn in range(min(2, NCH)):
                pidx = half * 2 + nn
                # out = psum * r + bias
                nc.vector.scalar_tensor_tensor(
                    o_t[:, nn * 512:(nn + 1) * 512], ppous[pidx][:], r128,
                    bias_rep[:, pidx * 512:(pidx + 1) * 512],
                    op0=mybir.AluOpType.mult, op1=mybir.AluOpType.add)
            nc.gpsimd.dma_start(
                o_ap[pm0:pm0 + 128, half * 1024:half * 1024 + w_out],
                o_t[:, 0:w_out])

    prev = None
    for i in range(MT):
        m0 = i * 128
        xq_t = xq_pool.tile([128, KT * 128], BF16, name=f"xq_{i}", tag="xq")
        for g in range(NKG):
            xs = io_pool.tile([128, KG * 128], F32, name=f"xs_{i}_{g}", tag="io")
            src = xt_ap[g * KG * 128:(g + 1) * KG * 128, m0:m0 + 128]
            nc.sync.dma_start(
                xs[:].rearrange("p (kt mj) -> p kt mj", kt=KG),
                src.rearrange("(kt p) mj -> p kt mj", p=128))
            # xs = round_to_int(x*s), in place: magic-add rounds half-to-even
            nc.scalar.activation(xs[:], xs[:],
                                 mybir.ActivationFunctionType.Identity,
                                 bias=magic128[:], scale=scale128)
            # xq = min(xs - magic, 127) -> bf16 (>= -128 by construction)
            nc.vector.tensor_scalar(
                xq_t[:, g * KG * 128:(g + 1) * KG * 128], xs[:],
                MAGIC, 127.0,
                op0=mybir.AluOpType.subtract, op1=mybir.AluOpType.min)

        if prev is not None:
            drain(*prev)

        pous = [po.tile([128, 512], F32, name=f"po_{i}_{n}", tag="po")
                for n in range(NCH)]
        for k in range(KT):
            lhsT = xq_t[:, k * 128:(k + 1) * 128]
            for n in range(NCH):
                nc.tensor.matmul(
                    pous[n][:], lhsT,
                    wqT[:, k * n_sh + n * 512: k * n_sh + (n + 1) * 512],
                    start=(k == 0), stop=(k == KT - 1))
        prev = (i, pous)

    drain(*prev)
    po_pool.__exit__(None, None, None)


def kernel(**inputs):
    x = np.ascontiguousarray(inputs["input"], dtype=np.float32).reshape(M, IN_F)
    w = np.ascontiguousarray(inputs["weight"], dtype=np.float32)
    b = np.ascontiguousarray(inputs["bias"], dtype=np.float32)

    xT = np.ascontiguousarray(x.T)            # [IN_F, M]
    wT = np.ascontiguousarray(w.T)            # [IN_F, OUT_F]

    if "nc" not in _CACHE:
        _CACHE["nc"] = _build()
    nc = _CACHE["nc"]

    in_maps = []
    for c in range(N_CORES):
        in_maps.append({
            "xt": xT,
            "x1": np.ascontiguousarray(xT[:, c * M_SL:(c + 1) * M_SL]),
            "w": np.ascontiguousarray(wT[:, c * N_SH:(c + 1) * N_SH]),
            "b": b[c * N_SH:(c + 1) * N_SH].reshape(1, N_SH),
        })
    res = bass_utils.run_bass_kernel_spmd(nc, in_maps,
                                          core_ids=list(range(N_CORES)))
    _CACHE["last_results"] = res
    _CACHE["last_in_maps"] = in_maps
    out = np.concatenate([r["o"] for r in res.results], axis=1)
    return out.reshape(B, S, OUT_F)


# revision 20
# speedup vs baseline: 1.5760x; 1.3494x over previous
"""BitLinear 1.58-bit (nn_BitLinear158) Trainium2 kernel, 8-core tensor-parallel.

Math (must match reference):
  gamma_x = max(max|x|, eps); s = 128/gamma_x; xq = clip(round(x*s), -128, 127)
  gamma_w = max(mean|w|, eps); wq = clip(round(w/gamma_w), -1, 1)  (ternary)
  out = (xq @ wq.T) * (gamma_w / s) + bias

Key facts exploited:
  - xq in [-128,127] and wq in {-1,0,1} are exact in bf16; products and all
    PSUM partial sums are integers < 2^20, exact in fp32 => the GEMM runs at
    full bf16 PE rate and is bit-identical to the fp32 reference einsum.
  - wq = 1[w > 0.5*gamma_w] - 1[w < -0.5*gamma_w] (no division / round).
  - round-half-even via the fp32 magic constant 1.5*2^23 (valid for |v|<=2^22).

v2 vs baseline:
  - x and w are handed over TRANSPOSED (k-major) by the host wrapper, so
    both GEMM operands arrive with the contraction dim on partitions and the
    PE never runs a transpose: it executes matmuls only.
  - Redundant InstLdweights (same stationary tile as the previous matmul)
    are rewritten to no-ops post-scheduling: 4 matmuls (n-chunks) share one
    weight load.
  - One AllGather replaces the two AllReduces for (sum|w|, max|x|).
  - All 8 PSUM banks double-buffer the accumulation groups.

Sharding: weight/bias split over out_features (16384 -> 8 x 2048), x
replicated; per-core GEMM [8192,4096]x[4096,2048].
"""

from contextlib import ExitStack

import numpy as np

import concourse.bass as bass
import concourse.mybir as mybir
import concourse.tile as tile
from concourse import bass_utils
from concourse.masks import make_identity
from concourse.vector_clock import ScopedClock

# ---------------------------------------------------------------------------
# Workaround: this walrus build rejects instructions carrying >1-2 sync wait
# commands. Tile's tail drain (emitted after tile_legalize) aggregates one
# wait per outstanding proc onto a single InstDrain and so escapes the
# wait-count legalization. Redistribute its waits across a chain of NO-queue
# nops (same sequencer => program order preserves the barrier semantics).
# ---------------------------------------------------------------------------
_MAX_WAITS = 1


def _patched_drain_and_barrier(self, tick_clock, wait_clock):
    nc = self.nc
    probe = nc.sync.nop()
    wait_clock.add_sem_waits(probe.ins, ScopedClock({None: tick_clock.global_clock}))
    si = probe.ins.sync_info
    waits = list(si.on_wait) if si is not None and si.on_wait else []
    ups = list(si.on_update) if si is not None and si.on_update else []
    probe.ins.sync_info = mybir.SyncInfo(on_wait=waits[:_MAX_WAITS], on_update=ups)
    rest = waits[_MAX_WAITS:]
    while rest:
        n2 = nc.sync.nop()
        n2.ins.sync_info = mybir.SyncInfo(on_wait=rest[:_MAX_WAITS], on_update=[])
        rest = rest[_MAX_WAITS:]

    nc.sync.drain()

    nc.all_engine_barrier()
    assert self.sems is not None
    popped = nc._tile_sem_poison_stack.pop()
    assert popped is self._sem_poison
    nc.clear_and_free_semaphores(list(self.sems.allocated().values()))
    nc.all_engine_barrier()


tile.TileContext._drain_and_barrier = _patched_drain_and_barrier

_nop_counter = [0]


def _legalize_waits(nc):
    """Split >_MAX_WAITS sync waits per instruction onto same-engine nops
    inserted immediately before (per-engine program order => semantics kept)."""
    for f in nc.m.functions:
        for blk in f.blocks:
            out = []
            changed = False
            for inst in blk.instructions:
                si = getattr(inst, "sync_info", None)
                waits = list(si.on_wait) if si is not None and si.on_wait else []
                if len(waits) > _MAX_WAITS and inst.engine != mybir.EngineType.Unassigned:
                    while len(waits) > _MAX_WAITS:
                        chunk, waits = waits[:_MAX_WAITS], waits[_MAX_WAITS:]
                        _nop_counter[0] += 1
                        out.append(mybir.InstNoOp(
                            name=f"waitnop-{_nop_counter[0]}",
                            engine=inst.engine, ins=[], outs=[],
                            sync_info=mybir.SyncInfo(on_wait=chunk, on_update=[]),
                        ))
                    inst.sync_info = mybir.SyncInfo(
                        on_wait=waits,
                        on_update=list(si.on_update) if si.on_update else [])
                    changed = True
                out.append(inst)
            if changed:
                blk.instructions = out


def _strip_mm_incs(nc, verbose=False):
    """Remove the vector-clock sem-inc from non-group-final matmuls (their
    only purpose is advancing the PE clock; consumers wait on group-end
    counts). All waits on that semaphore are remapped onto the kept-inc
    numbering, rounding up so no dependency is ever released early."""
    import collections

    for f in nc.m.functions:
        for blk in f.blocks:
            pe_insts = [i for i in blk.instructions
                        if i.engine == mybir.EngineType.PE]
            cnt = collections.Counter()
            for i in pe_insts:
                si = getattr(i, "sync_info", None)
                for u in (si.on_update if si is not None and si.on_update else []):
                    if u.update_mode == "sem-inc":
                        cnt[u.id] += 1
            if not cnt or cnt.most_common(1)[0][1] < 100:
                continue
            sem_id = cnt.most_common(1)[0][0]

            incs = []  # (inst, strippable)
            for i in pe_insts:
                si = getattr(i, "sync_info", None)
                ups = [u for u in (si.on_update if si is not None and si.on_update else [])
                       if u.id == sem_id]
                if not ups:
                    continue
                assert len(ups) == 1 and ups[0].update_value == 1
                strippable = (type(i).__name__ == "InstMatmult"
                              and getattr(i, "stop_tensor_calc", True) is False)
                incs.append([i, strippable])
            if incs:
                incs[-1][1] = False  # final inc must survive (drain waits total)

            kept_prefix = []
            k = 0
            for _, s in incs:
                if not s:
                    k += 1
                kept_prefix.append(k)
            kept_total = k

            def remap(v):
                if v <= 0:
                    return v
                vi = min(v, len(incs))
                nv = kept_prefix[vi - 1]
                if incs[vi - 1][1]:  # waited-on inc was stripped: round up
                    nv += 1
                assert nv <= kept_total
                return nv

            # rewrite waits everywhere, then strip updates
            nwait = nstrip = 0
            for i in blk.instructions:
                si = getattr(i, "sync_info", None)
                if si is None or not si.on_wait:
                    continue
                changed = False
                for w in si.on_wait:
                    if w.id == sem_id:
                        assert w.wait_mode == "sem-ge-imm", w.wait_mode
                        w.wait_value = remap(w.wait_value)
                        changed = True
                if changed:
                    nwait += 1
                    i.sync_info = mybir.SyncInfo(
                        on_wait=list(si.on_wait),
                        on_update=list(si.on_update) if si.on_update else [])
            for i, strippable in incs:
                if strippable:
                    si = i.sync_info
                    i.sync_info = mybir.SyncInfo(
                        on_wait=list(si.on_wait) if si.on_wait else [],
                        on_update=[u for u in si.on_update if u.id != sem_id])
                    nstrip += 1
            if verbose:
                print(f"_strip_mm_incs: stripped {nstrip} incs, "
                      f"remapped {nwait} waiters, kept {kept_total}")


def _ldw_key(inst):
    ap = inst.ins[0]
    return (str(ap.ap), ap.offset, str(ap.dtype), ap.memref)


def _dedup_ldweights(nc, verbose=False):
    """Rewrite InstLdweights that reload the stationary tile already resident
    in the PE array into no-ops (PE weight regs persist across matmuls; only
    transpose-mode matmuls clobber them)."""
    total = dropped = 0
    for f in nc.m.functions:
        for blk in f.blocks:
            out = []
            last = None
            changed = False
            for inst in blk.instructions:
                if inst.engine == mybir.EngineType.PE:
                    tn = type(inst).__name__
                    if tn == "InstLdweights":
                        total += 1
                        key = _ldw_key(inst)
                        if key == last:
                            dropped += 1
                            changed = True
                            si = inst.sync_info
                            has_sync = si is not None and (si.on_wait or si.on_update)
                            if has_sync:
                                _nop_counter[0] += 1
                                out.append(mybir.InstNoOp(
                                    name=f"ldwnop-{_nop_counter[0]}",
                                    engine=mybir.EngineType.PE, ins=[], outs=[],
                                    sync_info=si))
                            continue
                        last = key
                    elif tn == "InstMatmult":
                        if getattr(inst, "is_transpose", False):
                            last = None
                    elif tn in ("InstNoOp", "InstEventSemaphore", "InstDrain",
                                "InstRegisterMove", "InstUnconditionalBranch"):
                        pass
                    else:
                        last = None
                out.append(inst)
            if changed:
                blk.instructions = out
    if verbose:
        print(f"_dedup_ldweights: dropped {dropped}/{total}")
    return dropped, total


# ---------------------------------------------------------------------------

N_CORES = 8
B, S, IN_F, OUT_F = 4, 2048, 4096, 16384
M = B * S                    # 8192 rows of x
N_SH = OUT_F // N_CORES      # 2048 output features per core
M_SL = M // N_CORES          # per-core slice of x for the pass-1 max
EPS = 1e-5
MAGIC = 12582912.0           # 1.5 * 2^23: fp32 round-to-nearest-even trick
F32 = mybir.dt.float32
BF16 = mybir.dt.bfloat16

_CACHE = {}


def _build(collective=True, m=M, in_f=IN_F, n_sh=N_SH, m_sl=M_SL, out_f=OUT_F,
           postpasses=True, ablate=None):
    nc = bass.Bass("TRN2", target_bir_lowering=False, debug=False,
                   num_devices=N_CORES if collective else 1)
    xt_ap = nc.dram_tensor("xt", [in_f, m], F32, kind="ExternalInput").ap()
    x1_ap = nc.dram_tensor("x1", [in_f, m_sl], F32, kind="ExternalInput").ap()
    w_ap = nc.dram_tensor("w", [in_f, n_sh], F32, kind="ExternalInput").ap()
    b_ap = nc.dram_tensor("b", [1, n_sh], F32, kind="ExternalInput").ap()
    o_ap = nc.dram_tensor("o", [m, n_sh], F32, kind="ExternalOutput").ap()

    with tile.TileContext(nc) as tc:
        with ExitStack() as stack:
            _body(nc, tc, stack, xt_ap, x1_ap, w_ap, b_ap, o_ap,
                  collective=collective, m=m, in_f=in_f, n_sh=n_sh,
                  m_sl=m_sl, out_f=out_f, ablate=ablate)
    if postpasses:
        _dedup_ldweights(nc, verbose=True)
        _legalize_waits(nc)
    return nc


def _body(nc, tc, stack, xt_ap, x1_ap, w_ap, b_ap, o_ap, collective,
          m, in_f, n_sh, m_sl, out_f, ablate=None):
    KT = in_f // 128             # k-tiles
    MT = m // 128                # m-tiles
    NCH = n_sh // 512            # psum column chunks per m-tile
    KG = min(16, KT)             # k-tiles per staging DMA
    NKG = KT // KG               # staging DMAs per m-tile

    def pool(name, bufs, space="SBUF"):
        return stack.enter_context(
            tc.tile_pool(name=name, bufs=bufs, space=space))

    # --- persistent SBUF ---
    wq_pool = pool("wq", 1)
    # wqT layout: [128 k-part, KT * n_sh] bf16, k-tile major
    wqT = wq_pool.tile([128, KT * n_sh], BF16, name="wqT", tag="wqT")
    const_pool = pool("const", 1)
    ident_f32 = const_pool.tile([128, 128], F32, name="ident_f32", tag="if32")
    ones_row = const_pool.tile([1, 128], F32, name="ones_row", tag="ones")
    bias_rep = const_pool.tile([128, n_sh], BF16, name="bias_rep", tag="brep")
    scal128 = const_pool.tile([128, 4], F32, name="scal128", tag="scal128")
    magic128 = const_pool.tile([128, 1], F32, name="magic128", tag="magic")
    stats_pool = pool("stats", 1)
    wsums = stats_pool.tile([128, 2 * KT], F32, name="wsums", tag="wsums")
    xmaxs = stats_pool.tile([128, KT], F32, name="xmaxs", tag="xmaxs")
    stats2 = stats_pool.tile([128, 2], F32, name="stats2", tag="stats2")
    statsT_w = stats_pool.tile([1, 128], F32, name="statsT_w", tag="statsTw")
    statsT_x = stats_pool.tile([1, 128], F32, name="statsT_x", tag="statsTx")
    sc = stats_pool.tile([1, 12], F32, name="sc", tag="sc")
    ag = stats_pool.tile([1, 2 * N_CORES], F32, name="ag", tag="ag")

    # --- rotating SBUF ---
    io_pool = pool("io", 3)          # [128, 1024] f32 x staging (4 kt x 256 m)
    wio_pool = pool("wio", 3)        # [128, 1024] f32 prep staging (w / x1)
    neg_pool = pool("neg", 2)        # [128, 1024] bf16 scratch for w quantize
    xq_pool = pool("xq", 2)          # [128, KT*256] bf16 quantized m-pair slab
    out_pool = pool("outp", 3)       # [128, 512] f32 output staging
    bch_pool = pool("bch", 2)        # [1, 512] f32 bias chunks

    make_identity(nc, ident_f32[:])
    nc.gpsimd.memset(ones_row[:], 1.0)
    nc.gpsimd.memset(magic128[:], MAGIC)

    psum_prep = tc.tile_pool(name="psum_prep", bufs=2, space="PSUM")
    pp = psum_prep.__enter__()

    # ---------------- pass 1: |w| row sums + sliced max|x| ----------------
    # x responsibility for the global max is M-sharded across cores (each
    # core scans 1/8 of x = its x1 input, columns of xT); an AllReduce(max)
    # recovers the exact global max.
    nh = n_sh // 2
    for j in range(KT):
        for h in range(2):
            w_h = wio_pool.tile([128, 1024], F32, name=f"wh_{j}_{h}", tag="wio")
            nc.sync.dma_start(w_h[:, 0:nh],
                              w_ap[j * 128:(j + 1) * 128, h * nh:(h + 1) * nh])
            # |w| row-sum on the (otherwise idle) scalar engine: out=|w| is a
            # throwaway in-place write, accum_out catches the partition sum.
            nc.scalar.activation(w_h[:, 0:nh], w_h[:, 0:nh],
                                 mybir.ActivationFunctionType.Abs,
                                 accum_out=wsums[:, 2 * j + h:2 * j + h + 1])
    nc.vector.tensor_reduce(stats2[:, 0:1], wsums[:],
                            axis=mybir.AxisListType.X, op=mybir.AluOpType.add)

    for j in range(KT):
        x_h = wio_pool.tile([128, 1024], F32, name=f"xh1_{j}", tag="wio")
        nc.sync.dma_start(x_h[:, 0:m_sl], x1_ap[j * 128:(j + 1) * 128, :])
        nc.vector.tensor_reduce(xmaxs[:, j:j + 1], x_h[:, 0:m_sl],
                                axis=mybir.AxisListType.X,
                                op=mybir.AluOpType.max,
                                apply_absolute_value=True)
    nc.vector.tensor_reduce(stats2[:, 1:2], xmaxs[:],
                            axis=mybir.AxisListType.X, op=mybir.AluOpType.max)

    # cross-partition reductions via PE transpose
    st_ps_w = pp.tile([1, 128], F32, name="st_ps_w", tag="prep")
    nc.tensor.transpose(st_ps_w[:], stats2[:, 0:1], ident_f32[:])
    nc.vector.tensor_copy(statsT_w[:], st_ps_w[:])
    nc.vector.tensor_reduce(sc[0:1, 0:1], statsT_w[:],
                            axis=mybir.AxisListType.X, op=mybir.AluOpType.add)
    st_ps_x = pp.tile([1, 128], F32, name="st_ps_x", tag="prep")
    nc.tensor.transpose(st_ps_x[:], stats2[:, 1:2], ident_f32[:])
    nc.vector.tensor_copy(statsT_x[:], st_ps_x[:])
    nc.vector.tensor_reduce(sc[0:1, 1:2], statsT_x[:],
                            axis=mybir.AxisListType.X, op=mybir.AluOpType.max)

    if collective:
        dram_pool = pool("dram", 1, space="DRAM")
        cc_in = dram_pool.tile([1, 2], F32, name="cc_in", tag="cc_in")
        cc_out = dram_pool.tile([1, 2 * N_CORES], F32, name="cc_out",
                                tag="cc_out", addr_space="Shared")
        nc.gpsimd.dma_start(cc_in[:], sc[0:1, 0:2])
        nc.gpsimd.collective_compute(
            "AllGather", mybir.AluOpType.bypass,
            replica_groups=[list(range(N_CORES))],
            ins=[cc_in[:].opt()], outs=[cc_out[:].opt()],
        )
        nc.gpsimd.dma_start(ag[:], cc_out[:])
        # core-major [w0, x0, w1, x1, ...] -> strided views
        ag3 = ag[:].rearrange("p (c t) -> p t c", t=2)
        nc.vector.tensor_reduce(sc[0:1, 2:3], ag3[0:1, 0:1, :],
                                axis=mybir.AxisListType.X,
                                op=mybir.AluOpType.add)
        nc.vector.tensor_reduce(sc[0:1, 3:4], ag3[0:1, 1:2, :],
                                axis=mybir.AxisListType.X,
                                op=mybir.AluOpType.max)
        wsum_all = sc[0:1, 2:3]
        xmax_all = sc[0:1, 3:4]
        inv_cnt = 1.0 / (out_f * in_f)
    else:  # single-core sim variant: local stats stand in for global ones
        wsum_all = sc[0:1, 0:1]
        xmax_all = sc[0:1, 1:2]
        inv_cnt = 1.0 / (n_sh * in_f)

    # gamma_w = max(sum/count, eps) -> sc[0,8]
    nc.vector.tensor_scalar(sc[0:1, 8:9], wsum_all, inv_cnt, EPS,
                            op0=mybir.AluOpType.mult, op1=mybir.AluOpType.max)
    # thr = 0.5*gamma_w -> sc[0,4]; nthr -> sc[0,5]
    nc.vector.tensor_scalar(sc[0:1, 4:5], sc[0:1, 8:9], 0.5, None,
                            op0=mybir.AluOpType.mult)
    nc.vector.tensor_scalar(sc[0:1, 5:6], sc[0:1, 8:9], -0.5, None,
                            op0=mybir.AluOpType.mult)
    # gamma_x = max(xmax, eps) -> sc[0,3] slot
    nc.vector.tensor_scalar(sc[0:1, 3:4], xmax_all, EPS, None,
                            op0=mybir.AluOpType.max)
    # scale_x = 128 / gamma_x -> sc[0,6]
    nc.vector.reciprocal(sc[0:1, 6:7], sc[0:1, 3:4])
    nc.vector.tensor_scalar(sc[0:1, 6:7], sc[0:1, 6:7], 128.0, None,
                            op0=mybir.AluOpType.mult)
    # r = gamma_w * gamma_x / 128 -> sc[0,7]
    nc.vector.tensor_scalar(sc[0:1, 7:8], sc[0:1, 3:4], 1.0 / 128.0, None,
                            op0=mybir.AluOpType.mult)
    nc.vector.tensor_mul(sc[0:1, 7:8], sc[0:1, 7:8], sc[0:1, 8:9])

    # broadcast [thr, nthr, scale, r] to 128 partitions
    scb_ps = pp.tile([128, 4], F32, name="scb_ps", tag="prep")
    nc.tensor.matmul(scb_ps[:], ones_row[:], sc[0:1, 4:8], start=True, stop=True)
    nc.vector.tensor_copy(scal128[:, 0:4], scb_ps[:])
    thr128 = scal128[:, 0:1]
    nthr128 = scal128[:, 1:2]
    scale128 = scal128[:, 2:3]
    r128 = scal128[:, 3:4]

    # bias broadcast to 128 partitions
    for n in range(NCH):
        bch = bch_pool.tile([1, 512], F32, name=f"bch_{n}", tag="bch")
        nc.sync.dma_start(bch[:], b_ap[0:1, n * 512:(n + 1) * 512])
        b_ps = pp.tile([128, 512], F32, name=f"b_ps_{n}", tag="prep")
        nc.tensor.matmul(b_ps[:], ones_row[:], bch[:], start=True, stop=True)
        nc.vector.tensor_copy(bias_rep[:, n * 512:(n + 1) * 512], b_ps[:])
    psum_prep.__exit__(None, None, None)

    if ablate == "pass1_only":
        return
    # -------- quantize the weight shard (already k-major: no transpose) ----
    for j in range(KT):
        for h in range(2):
            w_h = wio_pool.tile([128, 1024], F32, name=f"wh2_{j}_{h}",
                                tag="wio")
            nc.sync.dma_start(w_h[:, 0:nh],
                              w_ap[j * 128:(j + 1) * 128, h * nh:(h + 1) * nh])
            neg = neg_pool.tile([128, 1024], BF16, name=f"neg_{j}_{h}",
                                tag="neg")
            nc.vector.tensor_scalar(neg[:, 0:nh], w_h[:, 0:nh], nthr128, None,
                                    op0=mybir.AluOpType.is_lt)
            # wq = (w > thr) - (w < -thr)   in {-1, 0, 1}, bf16
            nc.vector.scalar_tensor_tensor(
                wqT[:, j * n_sh + h * nh:j * n_sh + (h + 1) * nh],
                w_h[:, 0:nh], thr128, neg[:, 0:nh],
                op0=mybir.AluOpType.is_gt, op1=mybir.AluOpType.subtract)

    if ablate == "prep_only":
        return

    po_pool = tc.tile_pool(name="po", bufs=8, space="PSUM")
    po = po_pool.__enter__()

    # ---------------- main loop over m-tiles ----------------
    # The drain of m-tile i-1 is emitted AFTER the quantize of m-tile i:
    # the drain stt WAITS on the matmuls, and the DVE queue is strict FIFO,
    # so it must sit behind the quantize work feeding the next matmul group
    # or it head-of-line-blocks the whole pipeline. Output DMAs go on the
    # otherwise-idle gpsimd queue for the same reason (they wait on the
    # drain; SP must keep streaming x loads).
    def drain(pi, ppous):
        pm0 = pi * 128
        for n in range(NCH):
            o_t = out_pool.tile([128, 512], F32, name=f"ot_{pi}_{n}",
                                tag="outp")
            # out = psum * r + bias
            nc.vector.scalar_tensor_tensor(
                o_t[:], ppous[n][:], r128,
                bias_rep[:, n * 512:(n + 1) * 512],
                op0=mybir.AluOpType.mult, op1=mybir.AluOpType.add)
            nc.gpsimd.dma_start(
                o_ap[pm0:pm0 + 128, n * 512:(n + 1) * 512], o_t[:])

    static_xq = None
    if ablate == "noquant":
        static_xq = xq_pool.tile([128, KT * 256], BF16, name="xq_static",
                                 tag="xq")
        nc.vector.tensor_scalar(static_xq[:], wqT[:, 0:KT * 256], 1.0, None,
                                op0=mybir.AluOpType.mult)

    # m is processed in pairs of m-tiles (256 m per chunk) so every x DMA
    # line is 1 KiB — 512 B lines measured only ~125 GB/s effective.
    assert m % 256 == 0 and KT % 4 == 0
    prev = None
    for c in range(m // 256):
        m0 = c * 256
        if static_xq is not None:
            xq_t = static_xq
        else:
            xq_t = xq_pool.tile([128, KT * 256], BF16, name=f"xq_{c}",
                                tag="xq")
        xq3 = xq_t[:].rearrange("p (kt mj) -> p kt mj", mj=256)
        for g in range(KT // 4 if static_xq is None else 0):
            xs = io_pool.tile([128, 1024], F32, name=f"xs_{c}_{g}", tag="io")
            src = xt_ap[g * 512:(g + 1) * 512, m0:m0 + 256]
            nc.sync.dma_start(
                xs[:].rearrange("p (kt mj) -> p kt mj", kt=4),
                src.rearrange("(kt p) mj -> p kt mj", p=128))
            # xs = round_to_int(x*s), in place: magic-add rounds half-to-even
            nc.scalar.activation(xs[:], xs[:],
                                 mybir.ActivationFunctionType.Identity,
                                 bias=magic128[:], scale=scale128)
            xs3 = xs[:].rearrange("p (kt mj) -> p kt mj", mj=256)
            # xq = min(xs - magic, 127) -> bf16 (>= -128 by construction);
            # two strided writes split the pair-chunk into per-m-tile columns
            for t in range(2):
                nc.vector.tensor_scalar(
                    xq3[:, 4 * g:4 * (g + 1), t * 128:(t + 1) * 128],
                    xs3[:, :, t * 128:(t + 1) * 128],
                    MAGIC, 127.0,
                    op0=mybir.AluOpType.subtract, op1=mybir.AluOpType.min)

        for t in range(2):
            i = 2 * c + t
            if prev is not None:
                drain(*prev)
            pous = [po.tile([128, 512], F32, name=f"po_{i}_{n}", tag="po")
                    for n in range(NCH)]
            for k in range(KT):
                lhsT = xq_t[:, k * 256 + t * 128:k * 256 + (t + 1) * 128]
                for n in range(NCH):
                    nc.tensor.matmul(
                        pous[n][:], lhsT,
                        wqT[:, k * n_sh + n * 512: k * n_sh + (n + 1) * 512],
                        start=(k == 0), stop=(k == KT - 1))
            prev = (i, pous)

    drain(*prev)
    po_pool.__exit__(None, None, None)


def kernel(**inputs):
    x = np.ascontiguousarray(inputs["input"], dtype=np.float32).reshape(M, IN_F)
    w = np.ascontiguousarray(inputs["weight"], dtype=np.float32)
    b = np.ascontiguousarray(inputs["bias"], dtype=np.float32)

    xT = np.ascontiguousarray(x.T)            # [IN_F, M]
    wT = np.ascontiguousarray(w.T)            # [IN_F, OUT_F]

    if "nc" not in _CACHE:
        _CACHE["nc"] = _build()
    nc = _CACHE["nc"]

    in_maps = []
    for c in range(N_CORES):
        in_maps.append({
            "xt": xT,
            "x1": np.ascontiguousarray(xT[:, c * M_SL:(c + 1) * M_SL]),
            "w": np.ascontiguousarray(wT[:, c * N_SH:(c + 1) * N_SH]),
            "b": b[c * N_SH:(c + 1) * N_SH].reshape(1, N_SH),
        })
    res = bass_utils.run_bass_kernel_spmd(nc, in_maps,
                                          core_ids=list(range(N_CORES)))
    _CACHE["last_results"] = res
    _CACHE["last_in_maps"] = in_maps
    out = np.concatenate([r["o"] for r in res.results], axis=1)
    return out.reshape(B, S, OUT_F)
